# revision 16
# baseline (speedup 1.0000x reference)
"""Trainium2 Bass kernel for the MergeDNA window-local ToMe merge.

Problem (T=8192 tokens, D=256, N=8192, WINDOW=512, N_SELECT=128):
  k = l2norm(x @ Wk); per 512-token window: bipartite-match even (a) tokens
  to odd (b) tokens by cosine score, merge the top-128 a-tokens into their
  best b-match (size-weighted for x, plain sum for s), emit
  [unmerged-a (128 rows, order preserved); merged-b (256 rows)] per window.

Strategy (per core, sequence-parallel over windows, 2 windows/core):
  Stage A turns all data-dependent control into tiny on-chip artifacts:
    - dense rank of best-scores (compare matrix + row-reduce) -> top-128 mask
    - prefix-sum matmuls -> compaction positions for unmerged/selected rows
    - DRAM row-index vectors (int32) for the unmerged and selected a-rows
    - MselT [128,256]: one-hot dest b-row per selected a-row
  Stage B streams the [512, 8192] s-window once:
    - unmerged a-rows: indirect-DMA row gather -> straight DMA to out_s
      (zero compute)
    - selected a-rows: indirect-DMA row gather -> one K=128 matmul per
      [128,512] output tile scatters them onto b rows; a fused DVE
      tensor_tensor_reduce adds streamed b_s and accumulates row sums
  Stage C reuses the same one-hot matrices on [128,257] tiles for the
  size-weighted x merge.

The kernel function takes FULL inputs, shards rows across 8 NeuronCores
(windows are independent), and concatenates the per-core outputs.
"""

import numpy as np

import concourse.bass as bass
import concourse.mybir as mybir
from concourse import bacc
from concourse.tile import TileContext

F32 = mybir.dt.float32
I32 = mybir.dt.int32

T, D, NS = 8192, 256, 8192
W = 512                     # window size (tokens)
A = W // 2                  # tokens per side (a=even, b=odd) = 256
R = 128                     # N_SELECT: merged tokens per window
OUT_W = W - R               # output rows per window = 384
NW = T // W                 # 16 windows
NCORES = 8
WPC = NW // NCORES          # 2 windows per core
ROWS_PC = W * WPC           # 1024 input rows per core
OUT_PC = OUT_W * WPC        # 768 output rows per core
CHUNK = 1024                # columns of s per streamed chunk
NCH = NS // CHUNK           # 8 chunks
EPS = 1e-6
BIG = 65536.0

# debug toggles for HW bisection
import os as _os
DBG_OUT_ENGINE = _os.environ.get("K_OUT_ENGINE", "scalar")  # scalar|sync
DBG_TTR = _os.environ.get("K_TTR", "1") == "1"              # fused add+reduce
DBG_INDIRECT = _os.environ.get("K_INDIRECT", "1") == "1"    # indirect gathers


def build_program() -> bass.Bass:
    nc = bacc.Bacc("TRN2")

    x = nc.dram_tensor("x", [ROWS_PC, D], F32, kind="ExternalInput")
    s = nc.dram_tensor("s", [ROWS_PC, NS], F32, kind="ExternalInput")
    wk = nc.dram_tensor("wk", [D, D], F32, kind="ExternalInput")
    ident_d = nc.dram_tensor("ident", [128, 128], F32, kind="ExternalInput")
    iota_d = nc.dram_tensor("iota512", [128, 512], F32, kind="ExternalInput")
    iotam_d = nc.dram_tensor("iotam", [128, 256], F32, kind="ExternalInput")
    iotap_d = nc.dram_tensor("iotap", [A, 1], F32, kind="ExternalInput")
    sut_d = nc.dram_tensor("sut", [A, A], F32, kind="ExternalInput")
    ltm_d = nc.dram_tensor("ltm", [A, A], F32, kind="ExternalInput")
    onesc_d = nc.dram_tensor("ones_col", [128, 1], F32, kind="ExternalInput")
    onesr_d = nc.dram_tensor("ones_row", [1, 128], F32, kind="ExternalInput")

    out_x = nc.dram_tensor("out_x", [OUT_PC, D], F32, kind="ExternalOutput")
    out_s = nc.dram_tensor("out_s", [OUT_PC, NS], F32, kind="ExternalOutput")

    AL = mybir.AluOpType
    AX = mybir.AxisListType

    with TileContext(nc) as tc:
        with (
            tc.tile_pool(name="const", bufs=1) as cp,
            tc.tile_pool(name="work", bufs=2) as wp,
            tc.tile_pool(name="stream", bufs=4) as sp,
            tc.tile_pool(name="pss", bufs=4, space="PSUM") as pss,
            tc.tile_pool(name="psb", bufs=4, space="PSUM") as psb,
        ):
            # ---- constants ----
            ident = cp.tile([128, 128], F32, name="ident", tag="ident")
            nc.sync.dma_start(out=ident, in_=ident_d[:, :])
            iota = cp.tile([128, 512], F32, name="iota", tag="iota")
            nc.sync.dma_start(out=iota, in_=iota_d[:, :])
            iotam = cp.tile([128, 256], F32, name="iotam", tag="iotam")
            nc.sync.dma_start(out=iotam, in_=iotam_d[:, :])
            onesc = cp.tile([128, 1], F32, name="onesc", tag="onesc")
            nc.sync.dma_start(out=onesc, in_=onesc_d[:, :])
            onesr = cp.tile([1, 128], F32, name="onesr", tag="onesr")
            nc.sync.dma_start(out=onesr, in_=onesr_d[:, :])
            sut, ltm, wk_sb, iotap = [], [], [], []
            for kt in range(2):
                t1 = cp.tile([128, A], F32, name=f"sut{kt}", tag=f"sut{kt}")
                nc.sync.dma_start(out=t1, in_=sut_d[128 * kt:128 * (kt + 1), :])
                sut.append(t1)
                t2 = cp.tile([128, A], F32, name=f"ltm{kt}", tag=f"ltm{kt}")
                nc.sync.dma_start(out=t2, in_=ltm_d[128 * kt:128 * (kt + 1), :])
                ltm.append(t2)
                t3 = cp.tile([128, D], F32, name=f"wk{kt}", tag=f"wk{kt}")
                nc.sync.dma_start(out=t3, in_=wk[128 * kt:128 * (kt + 1), :])
                wk_sb.append(t3)
                t4 = cp.tile([128, 1], F32, name=f"iotap{kt}", tag=f"iotap{kt}")
                nc.sync.dma_start(out=t4, in_=iotap_d[128 * kt:128 * (kt + 1), :])
                iotap.append(t4)

            def stage_a(w):
                r0 = W * w          # input row base of this window


                # ================= stage A: selection =================
                xa, xb = [], []
                for kt in range(2):
                    rb = r0 + 256 * kt
                    ta = wp.tile([128, D], F32, name=f"xa{kt}", tag=f"xa{kt}")
                    nc.sync.dma_start(out=ta, in_=x[rb:rb + 256:2, :])
                    xa.append(ta)
                    tb = wp.tile([128, D], F32, name=f"xb{kt}", tag=f"xb{kt}")
                    nc.sync.dma_start(out=tb, in_=x[rb + 1:rb + 256:2, :])
                    xb.append(tb)
                yield

                # Transpose x to [din, token] layout (PE transpose by blocks).
                xaT, xbT = [], []
                for d in range(2):
                    tta = wp.tile([128, A], F32, name=f"xaT{d}", tag=f"xaT{d}")
                    ttb = wp.tile([128, A], F32, name=f"xbT{d}", tag=f"xbT{d}")
                    for kt in range(2):
                        pa = pss.tile([128, 128], F32, name="tp_a", tag="ps_small")
                        nc.tensor.transpose(
                            out=pa, in_=xa[kt][:, 128 * d:128 * (d + 1)],
                            identity=ident)
                        nc.vector.tensor_copy(tta[:, 128 * kt:128 * (kt + 1)], pa)
                        pb = pss.tile([128, 128], F32, name="tp_b", tag="ps_small")
                        nc.tensor.transpose(
                            out=pb, in_=xb[kt][:, 128 * d:128 * (d + 1)],
                            identity=ident)
                        nc.vector.tensor_copy(ttb[:, 128 * kt:128 * (kt + 1)], pb)
                    xaT.append(tta)
                    xbT.append(ttb)
                yield

                # kT = Wk.T-contraction: kT[dout, tok] = sum_din Wk[din,dout]*xT[din,tok]
                kaT, kbT = [], []
                for d in range(2):
                    for side, xT, lst in (("a", xaT, kaT), ("b", xbT, kbT)):
                        pk = pss.tile([128, A], F32, name=f"k{side}T_ps",
                                      tag="ps_small")
                        for kt in range(2):
                            nc.tensor.matmul(
                                pk, lhsT=wk_sb[kt][:, 128 * d:128 * (d + 1)],
                                rhs=xT[kt], start=(kt == 0), stop=(kt == 1))
                        ksb = wp.tile([128, A], F32, name=f"k{side}T{d}",
                                      tag=f"k{side}T{d}")
                        nc.scalar.copy(ksb, pk)
                        lst.append(ksb)
                yield

                # Per-token L2 norms -> normalized kT (cosine keys).
                kna, knb = [], []
                for side, kT, lst in (("a", kaT, kna), ("b", kbT, knb)):
                    sq = []
                    for d in range(2):
                        sqt = wp.tile([128, A], F32, name=f"sq{side}{d}",
                                      tag=f"kn{side}{d}")
                        nc.scalar.square(sqt, kT[d])
                        sq.append(sqt)
                    pssq = pss.tile([1, A], F32, name=f"ssq{side}", tag="ps_small")
                    for d in range(2):
                        nc.tensor.matmul(pssq, lhsT=onesc, rhs=sq[d],
                                         start=(d == 0), stop=(d == 1))
                    nrm = wp.tile([1, A], F32, name=f"nrm{side}", tag=f"nrm{side}")
                    nc.scalar.sqrt(nrm, pssq)
                    nc.vector.tensor_scalar_max(nrm, nrm, EPS)
                    inv = wp.tile([1, A], F32, name=f"inv{side}", tag=f"inv{side}")
                    nc.vector.reciprocal(inv, nrm)
                    pbc = pss.tile([128, A], F32, name=f"bc{side}", tag="ps_small")
                    nc.tensor.matmul(pbc, lhsT=onesr, rhs=inv, start=True,
                                     stop=True)
                    bcs = wp.tile([128, A], F32, name=f"bcs{side}",
                                  tag=f"bcs{side}")
                    nc.vector.tensor_copy(bcs, pbc)
                    for d in range(2):
                        kn = wp.tile([128, A], F32, name=f"kn{side}{d}",
                                     tag=f"kn{side}{d}")
                        nc.vector.tensor_tensor(out=kn, in0=kT[d], in1=bcs,
                                                op=AL.mult)
                        lst.append(kn)
                yield

                # scores[i, j] = kna[:, i] . knb[:, j]  -> [256 (2 tiles), 256]
                sc = []
                for m in range(2):
                    psc = pss.tile([128, A], F32, name="sc_ps", tag="ps_small")
                    for d in range(2):
                        nc.tensor.matmul(
                            psc, lhsT=kna[d][:, 128 * m:128 * (m + 1)],
                            rhs=knb[d], start=(d == 0), stop=(d == 1))
                    scs = wp.tile([128, A], F32, name=f"sc{m}", tag=f"sc{m}")
                    nc.scalar.copy(scs, psc)
                    sc.append(scs)
                yield

                # Row max (best score) + argmax (first max index = dst).
                best, dst = [], []
                for m in range(2):
                    bst = wp.tile([128, 1], F32, name=f"best{m}", tag=f"best{m}")
                    nc.vector.reduce_max(bst, sc[m], axis=AX.X)
                    am = wp.tile([128, A], F32, name="am", tag="am")
                    nc.vector.scalar_tensor_tensor(
                        out=am, in0=sc[m], scalar=bst[:, :1], in1=iotam,
                        op0=AL.is_ge, op1=AL.mult)
                    dmin = wp.tile([128, 1], F32, name="dmin", tag="dmin")
                    nc.vector.tensor_reduce(dmin, am, axis=AX.X, op=AL.min)
                    dstt = wp.tile([128, 1], F32, name=f"dst{m}", tag=f"dst{m}")
                    nc.vector.tensor_scalar_add(dstt, dmin, BIG)
                    best.append(bst)
                    dst.append(dstt)
                yield

                # Broadcast best across partitions: bcb[p, j] = best[j].
                bcb = wp.tile([128, A], F32, name="bcb", tag="bcb")
                for m in range(2):
                    pbb = pss.tile([128, 128], F32, name="bb_ps", tag="ps_small")
                    nc.tensor.transpose(
                        out=pbb, in_=best[m][:, :1].to_broadcast([128, 128]),
                        identity=ident)
                    nc.vector.tensor_copy(bcb[:, 128 * m:128 * (m + 1)], pbb)
                yield

                # Dense rank with index tie-break:
                # rank_i = #{j: best_j > best_i} + #{j < i: best_j == best_i}
                msel, unm = [], []
                for m in range(2):
                    eqlt = wp.tile([128, A], F32, name="eqlt", tag="eqlt")
                    nc.vector.scalar_tensor_tensor(
                        out=eqlt, in0=bcb, scalar=best[m][:, :1], in1=ltm[m],
                        op0=AL.is_equal, op1=AL.mult)
                    gt = wp.tile([128, A], F32, name="gt", tag="gt")
                    rank = wp.tile([128, 1], F32, name="rank", tag="rank")
                    nc.vector.scalar_tensor_tensor(
                        out=gt, in0=bcb, scalar=best[m][:, :1], in1=eqlt,
                        op0=AL.is_gt, op1=AL.add, accum_out=rank)
                    ms = wp.tile([128, 1], F32, name=f"msel{m}", tag=f"msel{m}")
                    nc.vector.tensor_scalar(ms, rank, float(R), None, op0=AL.is_lt)
                    un = wp.tile([128, 1], F32, name=f"unm{m}", tag=f"unm{m}")
                    nc.vector.tensor_scalar(un, rank, float(R), None, op0=AL.is_ge)
                    msel.append(ms)
                    unm.append(un)
                yield

                # Compaction positions (prefix counts) via strictly-upper matmul,
                # for the unmerged set and the selected set.
                pos, selpos = [], []
                for m in range(2):
                    pps = pss.tile([128, 1], F32, name="pos_ps", tag="ps_small")
                    for kt in range(2):
                        nc.tensor.matmul(pps,
                                         lhsT=sut[kt][:, 128 * m:128 * (m + 1)],
                                         rhs=unm[kt], start=(kt == 0),
                                         stop=(kt == 1))
                    pst = wp.tile([128, 1], F32, name=f"pos{m}", tag=f"pos{m}")
                    nc.vector.tensor_copy(pst, pps)
                    pos.append(pst)
                    sps = pss.tile([128, 1], F32, name="spos_ps", tag="ps_small")
                    for kt in range(2):
                        nc.tensor.matmul(sps,
                                         lhsT=sut[kt][:, 128 * m:128 * (m + 1)],
                                         rhs=msel[kt], start=(kt == 0),
                                         stop=(kt == 1))
                    spt = wp.tile([128, 1], F32, name=f"selpos{m}",
                                  tag=f"selpos{m}")
                    nc.vector.tensor_copy(spt, sps)
                    selpos.append(spt)
                yield

                # One-hot compaction matrices (K-side layout).
                punmT, selT = [], []
                for kt in range(2):
                    pu = wp.tile([128, R], F32, name=f"punmT{kt}",
                                 tag=f"punmT{kt}")
                    nc.vector.tensor_scalar(pu, iota[:, :R], pos[kt][:, :1],
                                            unm[kt][:, :1], op0=AL.is_equal,
                                            op1=AL.mult)
                    punmT.append(pu)
                    st = wp.tile([128, R], F32, name=f"selT{kt}",
                                 tag=f"selT{kt}")
                    nc.vector.tensor_scalar(st, iota[:, :R], selpos[kt][:, :1],
                                            msel[kt][:, :1], op0=AL.is_equal,
                                            op1=AL.mult)
                    selT.append(st)
                yield

                # dst per selected row (in compact order) -> one-hot MselT.
                pds = pss.tile([128, 1], F32, name="pds", tag="ps_small")
                for kt in range(2):
                    nc.tensor.matmul(pds, lhsT=selT[kt], rhs=dst[kt],
                                     start=(kt == 0), stop=(kt == 1))
                dstsel = wp.tile([128, 1], F32, name="dstsel", tag="dstsel")
                nc.vector.tensor_copy(dstsel, pds)
                mselT = wp.tile([128, A], F32, name="mselT", tag="mselT")
                nc.vector.tensor_scalar(mselT, iota[:, :A], dstsel[:, :1], None,
                                        op0=AL.is_equal)

                # DRAM row-index vectors (int32) for the indirect gathers.
                rows_i = []
                for mats, nmv in ((punmT, "unmrows"), (selT, "srcrows")):
                    pix = pss.tile([128, 1], F32, name="pix", tag="ps_small")
                    for kt in range(2):
                        nc.tensor.matmul(pix, lhsT=mats[kt], rhs=iotap[kt],
                                         start=(kt == 0), stop=(kt == 1))
                    rf = wp.tile([128, 1], F32, name=f"{nmv}_f", tag=f"{nmv}_f")
                    # DRAM row of a-token i is r0 + 2i
                    nc.vector.tensor_scalar(rf, pix, 2.0, float(r0),
                                            op0=AL.mult, op1=AL.add)
                    ri = wp.tile([128, 1], I32, name=f"{nmv}_i", tag=f"{nmv}_i")
                    nc.vector.tensor_copy(ri, rf)
                    rows_i.append(ri)
                unmrows, srcrows = rows_i
                yield
                win[w] = dict(xa=xa, xb=xb, mselT=mselT, punmT=punmT,
                              selT=selT, unmrows=unmrows, srcrows=srcrows)
                yield

            win = {}
            gens = [stage_a(w) for w in range(WPC)]
            alive = list(gens)
            while alive:
                alive = [g for g in alive if next(g, StopIteration) is None]


            for w in range(WPC):
                r0 = W * w
                o0 = OUT_W * w
                st = win[w]
                xa, xb = st["xa"], st["xb"]
                mselT, punmT, selT = st["mselT"], st["punmT"], st["selT"]
                unmrows, srcrows = st["unmrows"], st["srcrows"]

                # ================= stage B: stream s =================
                bsnp = []
                for m in range(2):
                    bsnp.append(wp.tile([128, (CHUNK // 512) * NCH], F32, name=f"bsnp{m}",
                                        tag=f"bsnp{m}"))
                ssp = wp.tile([128, NCH], F32, name="ssp", tag="ssp")

                for c in range(NCH):
                    c0 = CHUNK * c
                    unm_t = sp.tile([128, CHUNK], F32, name="unm_t", tag="unm_t", bufs=2)
                    src_t = sp.tile([128, CHUNK], F32, name="src_t", tag="src_t", bufs=3)
                    if DBG_INDIRECT:
                        nc.gpsimd.indirect_dma_start(
                            out=unm_t, out_offset=None, in_=s[:, :],
                            in_offset=bass.IndirectOffsetOnAxis(ap=unmrows[:, :1],
                                                                axis=0),
                            element_offset=c0)
                        nc.gpsimd.indirect_dma_start(
                            out=src_t, out_offset=None, in_=s[:, :],
                            in_offset=bass.IndirectOffsetOnAxis(ap=srcrows[:, :1],
                                                                axis=0),
                            element_offset=c0)
                    else:
                        nc.sync.dma_start(out=unm_t,
                                          in_=s[r0:r0 + 256:2, c0:c0 + CHUNK])
                        nc.sync.dma_start(out=src_t,
                                          in_=s[r0 + 1:r0 + 256:2, c0:c0 + CHUNK])
                    # unmerged rows go straight out
                    _oeng = nc.scalar if DBG_OUT_ENGINE == "scalar" else nc.sync
                    _oeng.dma_start(out=out_s[o0:o0 + R, c0:c0 + CHUNK],
                                    in_=unm_t)
                    # selected-row sizes (row-sum partials)
                    nc.vector.reduce_sum(ssp[:, c:c + 1], src_t, axis=AX.X)

                    bs_t = []
                    for m in range(2):
                        rb = r0 + 256 * m
                        tb = sp.tile([128, CHUNK], F32, name=f"bst{m}",
                                     tag=f"bst{m}", bufs=8)
                        nc.sync.dma_start(out=tb,
                                          in_=s[rb + 1:rb + 256:2, c0:c0 + CHUNK])
                        bs_t.append(tb)

                    for m in range(2):
                        outb = sp.tile([128, CHUNK], F32, name=f"outb{m}",
                                       tag=f"outb{m}", bufs=3)
                        for h in range(CHUNK // 512):
                            hs = 512 * h
                            pB = psb.tile([128, 512], F32, name="pB", tag="pB")
                            nc.tensor.matmul(
                                pB, lhsT=mselT[:, 128 * m:128 * (m + 1)],
                                rhs=src_t[:, hs:hs + 512], start=True, stop=True)
                            if DBG_TTR:
                                # fused (pB + 0) + bs with row-sum accumulator;
                                # NOTE tensor_tensor_reduce hard-crashes TRN2
                                # here — scalar_tensor_tensor is the working
                                # fused form.
                                nc.vector.scalar_tensor_tensor(
                                    out=outb[:, hs:hs + 512], in0=pB,
                                    scalar=0.0, in1=bs_t[m][:, hs:hs + 512],
                                    op0=AL.add, op1=AL.add,
                                    accum_out=bsnp[m][:, (CHUNK // 512) * c + h:(CHUNK // 512) * c + h + 1])
                            else:
                                nc.vector.tensor_tensor(
                                    out=outb[:, hs:hs + 512], in0=pB,
                                    in1=bs_t[m][:, hs:hs + 512], op=AL.add)
                                nc.vector.reduce_sum(
                                    bsnp[m][:, (CHUNK // 512) * c + h:
                                            (CHUNK // 512) * c + h + 1],
                                    outb[:, hs:hs + 512], axis=AX.X)
                        ro = o0 + R + 128 * m
                        _oeng.dma_start(out=out_s[ro:ro + 128, c0:c0 + CHUNK],
                                        in_=outb)

                # ================= stage C: x merge =================
                srcsize = wp.tile([128, 1], F32, name="srcsize", tag="srcsize")
                nc.vector.reduce_sum(srcsize, ssp, axis=AX.X)
                bsnew = []
                for m in range(2):
                    bn = wp.tile([128, 1], F32, name=f"bsnew{m}", tag=f"bsnew{m}")
                    nc.vector.reduce_sum(bn, bsnp[m], axis=AX.X)
                    bsnew.append(bn)

                # selected x rows (compact order), weighted by size
                psx = pss.tile([128, D], F32, name="psx", tag="ps_small")
                for kt in range(2):
                    nc.tensor.matmul(psx, lhsT=selT[kt], rhs=xa[kt],
                                     start=(kt == 0), stop=(kt == 1))
                wsrc = wp.tile([128, D + 1], F32, name="wsrc", tag="wsrc")
                nc.scalar.mul(wsrc[:, :D], psx, srcsize[:, :1])
                nc.vector.tensor_copy(wsrc[:, D:D + 1], srcsize)

                # unmerged-a rows of out_x: plain gather via punmT
                pxa = pss.tile([128, D], F32, name="pxa", tag="ps_small")
                for kt in range(2):
                    nc.tensor.matmul(pxa, lhsT=punmT[kt], rhs=xa[kt],
                                     start=(kt == 0), stop=(kt == 1))
                outxa = wp.tile([128, D], F32, name="outxa", tag="outxa")
                nc.scalar.copy(outxa, pxa)
                (nc.scalar if DBG_OUT_ENGINE == "scalar" else nc.sync).dma_start(
                    out=out_x[o0:o0 + R, :], in_=outxa)

                # merged-b rows: (xb*b_size + Msel @ (srcx*srcsize)) / b_size_new
                for m in range(2):
                    pXB = psb.tile([128, D + 1], F32, name="pB", tag="pB")
                    nc.tensor.matmul(pXB,
                                     lhsT=mselT[:, 128 * m:128 * (m + 1)],
                                     rhs=wsrc, start=True, stop=True)
                    # b_size = b_size_new - (scattered src sizes)
                    bsz = wp.tile([128, 1], F32, name="bsz", tag="bsz")
                    nc.vector.tensor_tensor(out=bsz, in0=bsnew[m],
                                            in1=pXB[:, D:D + 1], op=AL.subtract)
                    wxb = wp.tile([128, D], F32, name="wxb", tag="wxb")
                    nc.scalar.mul(wxb, xb[m], bsz[:, :1])
                    numb = wp.tile([128, D], F32, name="numb", tag="numb")
                    nc.vector.tensor_tensor(out=numb, in0=pXB[:, :D], in1=wxb,
                                            op=AL.add)
                    den = wp.tile([128, 1], F32, name="den", tag="den")
                    nc.vector.tensor_scalar_max(den, bsnew[m], EPS)
                    rec = wp.tile([128, 1], F32, name="rec", tag="rec")
                    nc.vector.reciprocal(rec, den)
                    outxb = wp.tile([128, D], F32, name="outxb", tag="outxb")
                    nc.scalar.mul(outxb, numb, rec[:, :1])
                    ro = o0 + R + 128 * m
                    (nc.scalar if DBG_OUT_ENGINE == "scalar" else nc.sync).dma_start(
                        out=out_x[ro:ro + 128, :], in_=outxb)

    nc.compile()
    return nc


def host_constants() -> dict[str, np.ndarray]:
    i = np.arange(A, dtype=np.float32)
    return {
        "ident": np.eye(128, dtype=np.float32),
        "iota512": np.broadcast_to(np.arange(512, dtype=np.float32),
                                   (128, 512)).copy(),
        "iotam": np.broadcast_to(np.arange(256, dtype=np.float32) - BIG,
                                 (128, 256)).copy(),
        "iotap": i[:, None].copy(),
        # sut[j, i] = 1 if j < i  (strictly upper): prefix-count lhsT
        "sut": (i[:, None] < i[None, :]).astype(np.float32),
        # ltm[i, j] = 1 if j < i  (strictly lower): tie-break mask
        "ltm": (i[None, :] < i[:, None]).astype(np.float32),
        "ones_col": np.ones((128, 1), dtype=np.float32),
        "ones_row": np.ones((1, 128), dtype=np.float32),
    }


_CACHED_NC = None


def kernel(x: np.ndarray, s: np.ndarray, Wk: np.ndarray):
    from concourse.bass_utils import run_bass_kernel_spmd

    global _CACHED_NC
    if _CACHED_NC is None:
        _CACHED_NC = build_program()
    nc = _CACHED_NC

    x = np.ascontiguousarray(np.asarray(x, dtype=np.float32))
    s = np.ascontiguousarray(np.asarray(s, dtype=np.float32))
    Wk = np.ascontiguousarray(np.asarray(Wk, dtype=np.float32))
    consts = host_constants()

    in_maps = []
    for c in range(NCORES):
        r = ROWS_PC * c
        in_maps.append({
            "x": x[r:r + ROWS_PC],
            "s": s[r:r + ROWS_PC],
            "wk": Wk,
            **consts,
        })

    res = run_bass_kernel_spmd(nc, in_maps, list(range(NCORES)))
    new_x = np.concatenate([res.results[c]["out_x"] for c in range(NCORES)],
                           axis=0)
    new_s = np.concatenate([res.results[c]["out_s"] for c in range(NCORES)],
                           axis=0)
    return new_x, new_s


# revision 17
# speedup vs baseline: 1.1793x; 1.1793x over previous
"""Trainium2 Bass kernel for the MergeDNA window-local ToMe merge.

Problem (T=8192 tokens, D=256, N=8192, WINDOW=512, N_SELECT=128):
  k = l2norm(x @ Wk); per 512-token window: bipartite-match even (a) tokens
  to odd (b) tokens by cosine score, merge the top-128 a-tokens into their
  best b-match (size-weighted for x, plain sum for s), emit
  [unmerged-a (128 rows, order preserved); merged-b (256 rows)] per window.

Strategy (per core, sequence-parallel over windows, 2 windows/core):
  Stage A turns all data-dependent control into tiny on-chip artifacts:
    - dense rank of best-scores (compare matrix + row-reduce) -> top-128 mask
    - prefix-sum matmuls -> compaction positions for unmerged/selected rows
    - DRAM row-index vectors (int32) for the unmerged and selected a-rows
    - MselT [128,256]: one-hot dest b-row per selected a-row
  Stage B streams the [512, 8192] s-window once:
    - unmerged a-rows: indirect-DMA row gather -> straight DMA to out_s
      (zero compute)
    - selected a-rows: indirect-DMA row gather -> one K=128 matmul per
      [128,512] output tile scatters them onto b rows; a fused DVE
      tensor_tensor_reduce adds streamed b_s and accumulates row sums
  Stage C reuses the same one-hot matrices on [128,257] tiles for the
  size-weighted x merge.

The kernel function takes FULL inputs, shards rows across 8 NeuronCores
(windows are independent), and concatenates the per-core outputs.
"""

import numpy as np

import concourse.bass as bass
import concourse.mybir as mybir
from concourse import bacc
from concourse.tile import TileContext

F32 = mybir.dt.float32
I32 = mybir.dt.int32

T, D, NS = 8192, 256, 8192
W = 512                     # window size (tokens)
A = W // 2                  # tokens per side (a=even, b=odd) = 256
R = 128                     # N_SELECT: merged tokens per window
OUT_W = W - R               # output rows per window = 384
NW = T // W                 # 16 windows
NCORES = 8
WPC = NW // NCORES          # 2 windows per core
ROWS_PC = W * WPC           # 1024 input rows per core
OUT_PC = OUT_W * WPC        # 768 output rows per core
CHUNK = 1024                # columns of s per streamed chunk
NCH = NS // CHUNK           # 8 chunks
EPS = 1e-6
BIG = 65536.0

# debug toggles for HW bisection
import os as _os
DBG_OUT_ENGINE = _os.environ.get("K_OUT_ENGINE", "scalar")  # scalar|sync
DBG_TTR = _os.environ.get("K_TTR", "1") == "1"              # fused add+reduce
DBG_INDIRECT = _os.environ.get("K_INDIRECT", "1") == "1"    # indirect gathers


def build_program() -> bass.Bass:
    nc = bacc.Bacc("TRN2")

    x = nc.dram_tensor("x", [ROWS_PC, D], F32, kind="ExternalInput")
    s = nc.dram_tensor("s", [ROWS_PC, NS], F32, kind="ExternalInput")
    wk = nc.dram_tensor("wk", [D, D], F32, kind="ExternalInput")
    ident_d = nc.dram_tensor("ident", [128, 128], F32, kind="ExternalInput")
    iota_d = nc.dram_tensor("iota512", [128, 512], F32, kind="ExternalInput")
    iotam_d = nc.dram_tensor("iotam", [128, 256], F32, kind="ExternalInput")
    iotap_d = nc.dram_tensor("iotap", [A, 1], F32, kind="ExternalInput")
    sut_d = nc.dram_tensor("sut", [A, A], F32, kind="ExternalInput")
    ltm_d = nc.dram_tensor("ltm", [A, A], F32, kind="ExternalInput")
    onesc_d = nc.dram_tensor("ones_col", [128, 1], F32, kind="ExternalInput")
    onesr_d = nc.dram_tensor("ones_row", [1, 128], F32, kind="ExternalInput")

    out_x = nc.dram_tensor("out_x", [OUT_PC, D], F32, kind="ExternalOutput")
    out_s = nc.dram_tensor("out_s", [OUT_PC, NS], F32, kind="ExternalOutput")

    AL = mybir.AluOpType
    AX = mybir.AxisListType

    with TileContext(nc) as tc:
        with (
            tc.tile_pool(name="const", bufs=1) as cp,
            tc.tile_pool(name="work", bufs=2) as wp,
            tc.tile_pool(name="stream", bufs=4) as sp,
            tc.tile_pool(name="pss", bufs=4, space="PSUM") as pss,
            tc.tile_pool(name="psb", bufs=4, space="PSUM") as psb,
        ):
            # ---- constants ----
            ident = cp.tile([128, 128], F32, name="ident", tag="ident")
            nc.sync.dma_start(out=ident, in_=ident_d[:, :])
            iota = cp.tile([128, 512], F32, name="iota", tag="iota")
            nc.sync.dma_start(out=iota, in_=iota_d[:, :])
            iotam = cp.tile([128, 256], F32, name="iotam", tag="iotam")
            nc.sync.dma_start(out=iotam, in_=iotam_d[:, :])
            onesc = cp.tile([128, 1], F32, name="onesc", tag="onesc")
            nc.sync.dma_start(out=onesc, in_=onesc_d[:, :])
            onesr = cp.tile([1, 128], F32, name="onesr", tag="onesr")
            nc.sync.dma_start(out=onesr, in_=onesr_d[:, :])
            sut, ltm, wk_sb, iotap = [], [], [], []
            for kt in range(2):
                t1 = cp.tile([128, A], F32, name=f"sut{kt}", tag=f"sut{kt}")
                nc.sync.dma_start(out=t1, in_=sut_d[128 * kt:128 * (kt + 1), :])
                sut.append(t1)
                t2 = cp.tile([128, A], F32, name=f"ltm{kt}", tag=f"ltm{kt}")
                nc.sync.dma_start(out=t2, in_=ltm_d[128 * kt:128 * (kt + 1), :])
                ltm.append(t2)
                t3 = cp.tile([128, D], F32, name=f"wk{kt}", tag=f"wk{kt}")
                nc.sync.dma_start(out=t3, in_=wk[128 * kt:128 * (kt + 1), :])
                wk_sb.append(t3)
                t4 = cp.tile([128, 1], F32, name=f"iotap{kt}", tag=f"iotap{kt}")
                nc.sync.dma_start(out=t4, in_=iotap_d[128 * kt:128 * (kt + 1), :])
                iotap.append(t4)

            win = {}
            for w in range(WPC):
                r0 = W * w          # input row base of this window

                # ================= stage A: selection =================
                xa, xb = [], []
                for kt in range(2):
                    rb = r0 + 256 * kt
                    ta = wp.tile([128, D], F32, name=f"xa{kt}", tag=f"xa{kt}")
                    nc.sync.dma_start(out=ta, in_=x[rb:rb + 256:2, :])
                    xa.append(ta)
                    tb = wp.tile([128, D], F32, name=f"xb{kt}", tag=f"xb{kt}")
                    nc.sync.dma_start(out=tb, in_=x[rb + 1:rb + 256:2, :])
                    xb.append(tb)

                # Transpose x to [din, token] layout (PE transpose by blocks).
                xaT, xbT = [], []
                for d in range(2):
                    tta = wp.tile([128, A], F32, name=f"xaT{d}", tag=f"xaT{d}")
                    ttb = wp.tile([128, A], F32, name=f"xbT{d}", tag=f"xbT{d}")
                    for kt in range(2):
                        pa = pss.tile([128, 128], F32, name="tp_a", tag="ps_small")
                        nc.tensor.transpose(
                            out=pa, in_=xa[kt][:, 128 * d:128 * (d + 1)],
                            identity=ident)
                        nc.vector.tensor_copy(tta[:, 128 * kt:128 * (kt + 1)], pa)
                        pb = pss.tile([128, 128], F32, name="tp_b", tag="ps_small")
                        nc.tensor.transpose(
                            out=pb, in_=xb[kt][:, 128 * d:128 * (d + 1)],
                            identity=ident)
                        nc.vector.tensor_copy(ttb[:, 128 * kt:128 * (kt + 1)], pb)
                    xaT.append(tta)
                    xbT.append(ttb)

                # kT = Wk.T-contraction: kT[dout, tok] = sum_din Wk[din,dout]*xT[din,tok]
                kaT, kbT = [], []
                for d in range(2):
                    for side, xT, lst in (("a", xaT, kaT), ("b", xbT, kbT)):
                        pk = pss.tile([128, A], F32, name=f"k{side}T_ps",
                                      tag="ps_small")
                        for kt in range(2):
                            nc.tensor.matmul(
                                pk, lhsT=wk_sb[kt][:, 128 * d:128 * (d + 1)],
                                rhs=xT[kt], start=(kt == 0), stop=(kt == 1))
                        ksb = wp.tile([128, A], F32, name=f"k{side}T{d}",
                                      tag=f"k{side}T{d}")
                        nc.scalar.copy(ksb, pk)
                        lst.append(ksb)

                # Per-token L2 norms -> normalized kT (cosine keys).
                kna, knb = [], []
                for side, kT, lst in (("a", kaT, kna), ("b", kbT, knb)):
                    sq = []
                    for d in range(2):
                        sqt = wp.tile([128, A], F32, name=f"sq{side}{d}",
                                      tag=f"kn{side}{d}")
                        nc.scalar.square(sqt, kT[d])
                        sq.append(sqt)
                    pssq = pss.tile([1, A], F32, name=f"ssq{side}", tag="ps_small")
                    for d in range(2):
                        nc.tensor.matmul(pssq, lhsT=onesc, rhs=sq[d],
                                         start=(d == 0), stop=(d == 1))
                    nrm = wp.tile([1, A], F32, name=f"nrm{side}", tag=f"nrm{side}")
                    nc.scalar.sqrt(nrm, pssq)
                    nc.vector.tensor_scalar_max(nrm, nrm, EPS)
                    inv = wp.tile([1, A], F32, name=f"inv{side}", tag=f"inv{side}")
                    nc.vector.reciprocal(inv, nrm)
                    pbc = pss.tile([128, A], F32, name=f"bc{side}", tag="ps_small")
                    nc.tensor.matmul(pbc, lhsT=onesr, rhs=inv, start=True,
                                     stop=True)
                    bcs = wp.tile([128, A], F32, name=f"bcs{side}",
                                  tag=f"bcs{side}")
                    nc.vector.tensor_copy(bcs, pbc)
                    for d in range(2):
                        kn = wp.tile([128, A], F32, name=f"kn{side}{d}",
                                     tag=f"kn{side}{d}")
                        nc.vector.tensor_tensor(out=kn, in0=kT[d], in1=bcs,
                                                op=AL.mult)
                        lst.append(kn)

                # scores[i, j] = kna[:, i] . knb[:, j]  -> [256 (2 tiles), 256]
                sc = []
                for m in range(2):
                    psc = pss.tile([128, A], F32, name="sc_ps", tag="ps_small")
                    for d in range(2):
                        nc.tensor.matmul(
                            psc, lhsT=kna[d][:, 128 * m:128 * (m + 1)],
                            rhs=knb[d], start=(d == 0), stop=(d == 1))
                    scs = wp.tile([128, A], F32, name=f"sc{m}", tag=f"sc{m}")
                    nc.scalar.copy(scs, psc)
                    sc.append(scs)

                # Row max (best score) + argmax (first max index = dst).
                best, dst = [], []
                for m in range(2):
                    bst = wp.tile([128, 1], F32, name=f"best{m}", tag=f"best{m}")
                    nc.vector.reduce_max(bst, sc[m], axis=AX.X)
                    am = wp.tile([128, A], F32, name="am", tag="am")
                    nc.vector.scalar_tensor_tensor(
                        out=am, in0=sc[m], scalar=bst[:, :1], in1=iotam,
                        op0=AL.is_ge, op1=AL.mult)
                    dmin = wp.tile([128, 1], F32, name="dmin", tag="dmin")
                    nc.vector.tensor_reduce(dmin, am, axis=AX.X, op=AL.min)
                    dstt = wp.tile([128, 1], F32, name=f"dst{m}", tag=f"dst{m}")
                    nc.vector.tensor_scalar_add(dstt, dmin, BIG)
                    best.append(bst)
                    dst.append(dstt)

                # Broadcast best across partitions: bcb[p, j] = best[j].
                bcb = wp.tile([128, A], F32, name="bcb", tag="bcb")
                for m in range(2):
                    pbb = pss.tile([128, 128], F32, name="bb_ps", tag="ps_small")
                    nc.tensor.transpose(
                        out=pbb, in_=best[m][:, :1].to_broadcast([128, 128]),
                        identity=ident)
                    nc.vector.tensor_copy(bcb[:, 128 * m:128 * (m + 1)], pbb)

                # Dense rank with index tie-break:
                # rank_i = #{j: best_j > best_i} + #{j < i: best_j == best_i}
                msel, unm = [], []
                for m in range(2):
                    eqlt = wp.tile([128, A], F32, name="eqlt", tag="eqlt")
                    nc.vector.scalar_tensor_tensor(
                        out=eqlt, in0=bcb, scalar=best[m][:, :1], in1=ltm[m],
                        op0=AL.is_equal, op1=AL.mult)
                    gt = wp.tile([128, A], F32, name="gt", tag="gt")
                    rank = wp.tile([128, 1], F32, name="rank", tag="rank")
                    nc.vector.scalar_tensor_tensor(
                        out=gt, in0=bcb, scalar=best[m][:, :1], in1=eqlt,
                        op0=AL.is_gt, op1=AL.add, accum_out=rank)
                    ms = wp.tile([128, 1], F32, name=f"msel{m}", tag=f"msel{m}")
                    nc.vector.tensor_scalar(ms, rank, float(R), None, op0=AL.is_lt)
                    un = wp.tile([128, 1], F32, name=f"unm{m}", tag=f"unm{m}")
                    nc.vector.tensor_scalar(un, rank, float(R), None, op0=AL.is_ge)
                    msel.append(ms)
                    unm.append(un)

                # Compaction positions (prefix counts) via strictly-upper matmul,
                # for the unmerged set and the selected set.
                pos, selpos = [], []
                for m in range(2):
                    pps = pss.tile([128, 1], F32, name="pos_ps", tag="ps_small")
                    for kt in range(2):
                        nc.tensor.matmul(pps,
                                         lhsT=sut[kt][:, 128 * m:128 * (m + 1)],
                                         rhs=unm[kt], start=(kt == 0),
                                         stop=(kt == 1))
                    pst = wp.tile([128, 1], F32, name=f"pos{m}", tag=f"pos{m}")
                    nc.vector.tensor_copy(pst, pps)
                    pos.append(pst)
                    sps = pss.tile([128, 1], F32, name="spos_ps", tag="ps_small")
                    for kt in range(2):
                        nc.tensor.matmul(sps,
                                         lhsT=sut[kt][:, 128 * m:128 * (m + 1)],
                                         rhs=msel[kt], start=(kt == 0),
                                         stop=(kt == 1))
                    spt = wp.tile([128, 1], F32, name=f"selpos{m}",
                                  tag=f"selpos{m}")
                    nc.vector.tensor_copy(spt, sps)
                    selpos.append(spt)

                # One-hot compaction matrices (K-side layout).
                punmT, selT = [], []
                for kt in range(2):
                    pu = wp.tile([128, R], F32, name=f"punmT{kt}",
                                 tag=f"punmT{kt}")
                    nc.vector.tensor_scalar(pu, iota[:, :R], pos[kt][:, :1],
                                            unm[kt][:, :1], op0=AL.is_equal,
                                            op1=AL.mult)
                    punmT.append(pu)
                    st = wp.tile([128, R], F32, name=f"selT{kt}",
                                 tag=f"selT{kt}")
                    nc.vector.tensor_scalar(st, iota[:, :R], selpos[kt][:, :1],
                                            msel[kt][:, :1], op0=AL.is_equal,
                                            op1=AL.mult)
                    selT.append(st)

                # dst per selected row (in compact order) -> one-hot MselT.
                pds = pss.tile([128, 1], F32, name="pds", tag="ps_small")
                for kt in range(2):
                    nc.tensor.matmul(pds, lhsT=selT[kt], rhs=dst[kt],
                                     start=(kt == 0), stop=(kt == 1))
                dstsel = wp.tile([128, 1], F32, name="dstsel", tag="dstsel")
                nc.vector.tensor_copy(dstsel, pds)
                mselT = wp.tile([128, A], F32, name="mselT", tag="mselT")
                nc.vector.tensor_scalar(mselT, iota[:, :A], dstsel[:, :1], None,
                                        op0=AL.is_equal)

                # DRAM row-index vectors (int32) for the indirect gathers.
                rows_i = []
                for mats, nmv in ((punmT, "unmrows"), (selT, "srcrows")):
                    pix = pss.tile([128, 1], F32, name="pix", tag="ps_small")
                    for kt in range(2):
                        nc.tensor.matmul(pix, lhsT=mats[kt], rhs=iotap[kt],
                                         start=(kt == 0), stop=(kt == 1))
                    rf = wp.tile([128, 1], F32, name=f"{nmv}_f", tag=f"{nmv}_f")
                    # DRAM row of a-token i is r0 + 2i
                    nc.vector.tensor_scalar(rf, pix, 2.0, float(r0),
                                            op0=AL.mult, op1=AL.add)
                    ri = wp.tile([128, 1], I32, name=f"{nmv}_i", tag=f"{nmv}_i")
                    nc.vector.tensor_copy(ri, rf)
                    rows_i.append(ri)
                unmrows, srcrows = rows_i
                win[w] = dict(xa=xa, xb=xb, mselT=mselT, punmT=punmT,
                              selT=selT, unmrows=unmrows, srcrows=srcrows)

            for w in range(WPC):
                r0 = W * w
                o0 = OUT_W * w
                st = win[w]
                xa, xb = st["xa"], st["xb"]
                mselT, punmT, selT = st["mselT"], st["punmT"], st["selT"]
                unmrows, srcrows = st["unmrows"], st["srcrows"]

                # ================= stage B: stream s =================
                bsnp = []
                for m in range(2):
                    bsnp.append(wp.tile([128, (CHUNK // 512) * NCH], F32, name=f"bsnp{m}",
                                        tag=f"bsnp{m}"))
                ssp = wp.tile([128, NCH], F32, name="ssp", tag="ssp")

                for c in range(NCH):
                    c0 = CHUNK * c
                    unm_t = sp.tile([128, CHUNK], F32, name="unm_t", tag="unm_t", bufs=2)
                    src_t = sp.tile([128, CHUNK], F32, name="src_t", tag="src_t", bufs=3)
                    if DBG_INDIRECT:
                        nc.gpsimd.indirect_dma_start(
                            out=unm_t, out_offset=None, in_=s[:, :],
                            in_offset=bass.IndirectOffsetOnAxis(ap=unmrows[:, :1],
                                                                axis=0),
                            element_offset=c0)
                        nc.gpsimd.indirect_dma_start(
                            out=src_t, out_offset=None, in_=s[:, :],
                            in_offset=bass.IndirectOffsetOnAxis(ap=srcrows[:, :1],
                                                                axis=0),
                            element_offset=c0)
                    else:
                        nc.sync.dma_start(out=unm_t,
                                          in_=s[r0:r0 + 256:2, c0:c0 + CHUNK])
                        nc.sync.dma_start(out=src_t,
                                          in_=s[r0 + 1:r0 + 256:2, c0:c0 + CHUNK])
                    # unmerged rows go straight out
                    _oeng = nc.scalar if DBG_OUT_ENGINE == "scalar" else nc.sync
                    _oeng.dma_start(out=out_s[o0:o0 + R, c0:c0 + CHUNK],
                                    in_=unm_t)
                    # selected-row sizes (row-sum partials)
                    nc.vector.reduce_sum(ssp[:, c:c + 1], src_t, axis=AX.X)

                    bs_t = []
                    for m in range(2):
                        rb = r0 + 256 * m
                        tb = sp.tile([128, CHUNK], F32, name=f"bst{m}",
                                     tag=f"bst{m}", bufs=8)
                        nc.sync.dma_start(out=tb,
                                          in_=s[rb + 1:rb + 256:2, c0:c0 + CHUNK])
                        bs_t.append(tb)

                    for m in range(2):
                        outb = sp.tile([128, CHUNK], F32, name=f"outb{m}",
                                       tag=f"outb{m}", bufs=3)
                        for h in range(CHUNK // 512):
                            hs = 512 * h
                            pB = psb.tile([128, 512], F32, name="pB", tag="pB")
                            nc.tensor.matmul(
                                pB, lhsT=mselT[:, 128 * m:128 * (m + 1)],
                                rhs=src_t[:, hs:hs + 512], start=True, stop=True)
                            if DBG_TTR:
                                # fused (pB + 0) + bs with row-sum accumulator;
                                # NOTE tensor_tensor_reduce hard-crashes TRN2
                                # here — scalar_tensor_tensor is the working
                                # fused form.
                                nc.vector.scalar_tensor_tensor(
                                    out=outb[:, hs:hs + 512], in0=pB,
                                    scalar=0.0, in1=bs_t[m][:, hs:hs + 512],
                                    op0=AL.add, op1=AL.add,
                                    accum_out=bsnp[m][:, (CHUNK // 512) * c + h:(CHUNK // 512) * c + h + 1])
                            else:
                                nc.vector.tensor_tensor(
                                    out=outb[:, hs:hs + 512], in0=pB,
                                    in1=bs_t[m][:, hs:hs + 512], op=AL.add)
                                nc.vector.reduce_sum(
                                    bsnp[m][:, (CHUNK // 512) * c + h:
                                            (CHUNK // 512) * c + h + 1],
                                    outb[:, hs:hs + 512], axis=AX.X)
                        ro = o0 + R + 128 * m
                        _oeng.dma_start(out=out_s[ro:ro + 128, c0:c0 + CHUNK],
                                        in_=outb)

                # ================= stage C: x merge =================
                srcsize = wp.tile([128, 1], F32, name="srcsize", tag="srcsize")
                nc.vector.reduce_sum(srcsize, ssp, axis=AX.X)
                bsnew = []
                for m in range(2):
                    bn = wp.tile([128, 1], F32, name=f"bsnew{m}", tag=f"bsnew{m}")
                    nc.vector.reduce_sum(bn, bsnp[m], axis=AX.X)
                    bsnew.append(bn)

                # selected x rows (compact order), weighted by size
                psx = pss.tile([128, D], F32, name="psx", tag="ps_small")
                for kt in range(2):
                    nc.tensor.matmul(psx, lhsT=selT[kt], rhs=xa[kt],
                                     start=(kt == 0), stop=(kt == 1))
                wsrc = wp.tile([128, D + 1], F32, name="wsrc", tag="wsrc")
                nc.scalar.mul(wsrc[:, :D], psx, srcsize[:, :1])
                nc.vector.tensor_copy(wsrc[:, D:D + 1], srcsize)

                # unmerged-a rows of out_x: plain gather via punmT
                pxa = pss.tile([128, D], F32, name="pxa", tag="ps_small")
                for kt in range(2):
                    nc.tensor.matmul(pxa, lhsT=punmT[kt], rhs=xa[kt],
                                     start=(kt == 0), stop=(kt == 1))
                outxa = wp.tile([128, D], F32, name="outxa", tag="outxa")
                nc.scalar.copy(outxa, pxa)
                (nc.scalar if DBG_OUT_ENGINE == "scalar" else nc.sync).dma_start(
                    out=out_x[o0:o0 + R, :], in_=outxa)

                # merged-b rows: (xb*b_size + Msel @ (srcx*srcsize)) / b_size_new
                for m in range(2):
                    pXB = psb.tile([128, D + 1], F32, name="pB", tag="pB")
                    nc.tensor.matmul(pXB,
                                     lhsT=mselT[:, 128 * m:128 * (m + 1)],
                                     rhs=wsrc, start=True, stop=True)
                    # b_size = b_size_new - (scattered src sizes)
                    bsz = wp.tile([128, 1], F32, name="bsz", tag="bsz")
                    nc.vector.tensor_tensor(out=bsz, in0=bsnew[m],
                                            in1=pXB[:, D:D + 1], op=AL.subtract)
                    wxb = wp.tile([128, D], F32, name="wxb", tag="wxb")
                    nc.scalar.mul(wxb, xb[m], bsz[:, :1])
                    numb = wp.tile([128, D], F32, name="numb", tag="numb")
                    nc.vector.tensor_tensor(out=numb, in0=pXB[:, :D], in1=wxb,
                                            op=AL.add)
                    den = wp.tile([128, 1], F32, name="den", tag="den")
                    nc.vector.tensor_scalar_max(den, bsnew[m], EPS)
                    rec = wp.tile([128, 1], F32, name="rec", tag="rec")
                    nc.vector.reciprocal(rec, den)
                    outxb = wp.tile([128, D], F32, name="outxb", tag="outxb")
                    nc.scalar.mul(outxb, numb, rec[:, :1])
                    ro = o0 + R + 128 * m
                    (nc.scalar if DBG_OUT_ENGINE == "scalar" else nc.sync).dma_start(
                        out=out_x[ro:ro + 128, :], in_=outxb)

    nc.compile()
    return nc


def host_constants() -> dict[str, np.ndarray]:
    i = np.arange(A, dtype=np.float32)
    return {
        "ident": np.eye(128, dtype=np.float32),
        "iota512": np.broadcast_to(np.arange(512, dtype=np.float32),
                                   (128, 512)).copy(),
        "iotam": np.broadcast_to(np.arange(256, dtype=np.float32) - BIG,
                                 (128, 256)).copy(),
        "iotap": i[:, None].copy(),
        # sut[j, i] = 1 if j < i  (strictly upper): prefix-count lhsT
        "sut": (i[:, None] < i[None, :]).astype(np.float32),
        # ltm[i, j] = 1 if j < i  (strictly lower): tie-break mask
        "ltm": (i[None, :] < i[:, None]).astype(np.float32),
        "ones_col": np.ones((128, 1), dtype=np.float32),
        "ones_row": np.ones((1, 128), dtype=np.float32),
    }


_CACHED_NC = None


def kernel(x: np.ndarray, s: np.ndarray, Wk: np.ndarray):
    from concourse.bass_utils import run_bass_kernel_spmd

    global _CACHED_NC
    if _CACHED_NC is None:
        _CACHED_NC = build_program()
    nc = _CACHED_NC

    x = np.ascontiguousarray(np.asarray(x, dtype=np.float32))
    s = np.ascontiguousarray(np.asarray(s, dtype=np.float32))
    Wk = np.ascontiguousarray(np.asarray(Wk, dtype=np.float32))
    consts = host_constants()

    in_maps = []
    for c in range(NCORES):
        r = ROWS_PC * c
        in_maps.append({
            "x": x[r:r + ROWS_PC],
            "s": s[r:r + ROWS_PC],
            "wk": Wk,
            **consts,
        })

    res = run_bass_kernel_spmd(nc, in_maps, list(range(NCORES)))
    new_x = np.concatenate([res.results[c]["out_x"] for c in range(NCORES)],
                           axis=0)
    new_s = np.concatenate([res.results[c]["out_s"] for c in range(NCORES)],
                           axis=0)
    return new_x, new_s


# revision 18
# speedup vs baseline: 1.2296x; 1.0427x over previous
"""Trainium2 Bass kernel for the MergeDNA window-local ToMe merge.

Problem (T=8192 tokens, D=256, N=8192, WINDOW=512, N_SELECT=128):
  k = l2norm(x @ Wk); per 512-token window: bipartite-match even (a) tokens
  to odd (b) tokens by cosine score, merge the top-128 a-tokens into their
  best b-match (size-weighted for x, plain sum for s), emit
  [unmerged-a (128 rows, order preserved); merged-b (256 rows)] per window.

Strategy (per core, sequence-parallel over windows, 2 windows/core):
  Stage A turns all data-dependent control into tiny on-chip artifacts:
    - dense rank of best-scores (compare matrix + row-reduce) -> top-128 mask
    - prefix-sum matmuls -> compaction positions for unmerged/selected rows
    - DRAM row-index vectors (int32) for the unmerged and selected a-rows
    - MselT [128,256]: one-hot dest b-row per selected a-row
  Stage B streams the [512, 8192] s-window once:
    - unmerged a-rows: indirect-DMA row gather -> straight DMA to out_s
      (zero compute)
    - selected a-rows: indirect-DMA row gather -> one K=128 matmul per
      [128,512] output tile scatters them onto b rows; a fused DVE
      tensor_tensor_reduce adds streamed b_s and accumulates row sums
  Stage C reuses the same one-hot matrices on [128,257] tiles for the
  size-weighted x merge.

The kernel function takes FULL inputs, shards rows across 8 NeuronCores
(windows are independent), and concatenates the per-core outputs.
"""

import numpy as np

import concourse.bass as bass
import concourse.mybir as mybir
from concourse import bacc
from concourse.tile import TileContext

F32 = mybir.dt.float32
I32 = mybir.dt.int32

T, D, NS = 8192, 256, 8192
W = 512                     # window size (tokens)
A = W // 2                  # tokens per side (a=even, b=odd) = 256
R = 128                     # N_SELECT: merged tokens per window
OUT_W = W - R               # output rows per window = 384
NW = T // W                 # 16 windows
NCORES = 8
WPC = NW // NCORES          # 2 windows per core
ROWS_PC = W * WPC           # 1024 input rows per core
OUT_PC = OUT_W * WPC        # 768 output rows per core
CHUNK = 1024                # columns of s per streamed chunk
NCH = NS // CHUNK           # 8 chunks
EPS = 1e-6
BIG = 65536.0

# debug toggles for HW bisection
import os as _os
DBG_OUT_ENGINE = _os.environ.get("K_OUT_ENGINE", "scalar")  # scalar|sync
DBG_TTR = _os.environ.get("K_TTR", "1") == "1"              # fused add+reduce
DBG_INDIRECT = _os.environ.get("K_INDIRECT", "1") == "1"    # indirect gathers


def build_program() -> bass.Bass:
    nc = bacc.Bacc("TRN2")

    x = nc.dram_tensor("x", [ROWS_PC, D], F32, kind="ExternalInput")
    s = nc.dram_tensor("s", [ROWS_PC, NS], F32, kind="ExternalInput")
    wk = nc.dram_tensor("wk", [D, D], F32, kind="ExternalInput")
    ident_d = nc.dram_tensor("ident", [128, 128], F32, kind="ExternalInput")
    iota_d = nc.dram_tensor("iota512", [128, 512], F32, kind="ExternalInput")
    iotam_d = nc.dram_tensor("iotam", [128, 256], F32, kind="ExternalInput")
    iotap_d = nc.dram_tensor("iotap", [A, 1], F32, kind="ExternalInput")
    sut_d = nc.dram_tensor("sut", [A, A], F32, kind="ExternalInput")
    ltm_d = nc.dram_tensor("ltm", [A, A], F32, kind="ExternalInput")
    onesc_d = nc.dram_tensor("ones_col", [128, 1], F32, kind="ExternalInput")
    onesr_d = nc.dram_tensor("ones_row", [1, 128], F32, kind="ExternalInput")

    out_x = nc.dram_tensor("out_x", [OUT_PC, D], F32, kind="ExternalOutput")
    out_s = nc.dram_tensor("out_s", [OUT_PC, NS], F32, kind="ExternalOutput")

    AL = mybir.AluOpType
    AX = mybir.AxisListType

    with TileContext(nc) as tc:
        with (
            tc.tile_pool(name="const", bufs=1) as cp,
            tc.tile_pool(name="work", bufs=2) as wp,
            tc.tile_pool(name="stream", bufs=4) as sp,
            tc.tile_pool(name="pss", bufs=4, space="PSUM") as pss,
            tc.tile_pool(name="psb", bufs=4, space="PSUM") as psb,
        ):
            # ---- constants ----
            ident = cp.tile([128, 128], F32, name="ident", tag="ident")
            nc.sync.dma_start(out=ident, in_=ident_d[:, :])
            iota = cp.tile([128, 512], F32, name="iota", tag="iota")
            nc.sync.dma_start(out=iota, in_=iota_d[:, :])
            iotam = cp.tile([128, 256], F32, name="iotam", tag="iotam")
            nc.sync.dma_start(out=iotam, in_=iotam_d[:, :])
            onesc = cp.tile([128, 1], F32, name="onesc", tag="onesc")
            nc.sync.dma_start(out=onesc, in_=onesc_d[:, :])
            onesr = cp.tile([1, 128], F32, name="onesr", tag="onesr")
            nc.sync.dma_start(out=onesr, in_=onesr_d[:, :])
            sut, ltm, wk_sb, iotap = [], [], [], []
            for kt in range(2):
                t1 = cp.tile([128, A], F32, name=f"sut{kt}", tag=f"sut{kt}")
                nc.sync.dma_start(out=t1, in_=sut_d[128 * kt:128 * (kt + 1), :])
                sut.append(t1)
                t2 = cp.tile([128, A], F32, name=f"ltm{kt}", tag=f"ltm{kt}")
                nc.sync.dma_start(out=t2, in_=ltm_d[128 * kt:128 * (kt + 1), :])
                ltm.append(t2)
                t3 = cp.tile([128, D], F32, name=f"wk{kt}", tag=f"wk{kt}")
                nc.sync.dma_start(out=t3, in_=wk[128 * kt:128 * (kt + 1), :])
                wk_sb.append(t3)
                t4 = cp.tile([128, 1], F32, name=f"iotap{kt}", tag=f"iotap{kt}")
                nc.sync.dma_start(out=t4, in_=iotap_d[128 * kt:128 * (kt + 1), :])
                iotap.append(t4)

            win = {}
            for w in range(WPC):
                r0 = W * w          # input row base of this window

                # ================= stage A: selection =================
                xa, xb = [], []
                for kt in range(2):
                    rb = r0 + 256 * kt
                    ta = wp.tile([128, D], F32, name=f"xa{kt}", tag=f"xa{kt}")
                    nc.sync.dma_start(out=ta, in_=x[rb:rb + 256:2, :])
                    xa.append(ta)
                    tb = wp.tile([128, D], F32, name=f"xb{kt}", tag=f"xb{kt}")
                    nc.sync.dma_start(out=tb, in_=x[rb + 1:rb + 256:2, :])
                    xb.append(tb)

                # Transpose x to [din, token] layout (PE transpose by blocks).
                xaT, xbT = [], []
                for d in range(2):
                    tta = wp.tile([128, A], F32, name=f"xaT{d}", tag=f"xaT{d}")
                    ttb = wp.tile([128, A], F32, name=f"xbT{d}", tag=f"xbT{d}")
                    for kt in range(2):
                        pa = pss.tile([128, 128], F32, name="tp_a", tag="ps_small")
                        nc.tensor.transpose(
                            out=pa, in_=xa[kt][:, 128 * d:128 * (d + 1)],
                            identity=ident)
                        nc.vector.tensor_copy(tta[:, 128 * kt:128 * (kt + 1)], pa)
                        pb = pss.tile([128, 128], F32, name="tp_b", tag="ps_small")
                        nc.tensor.transpose(
                            out=pb, in_=xb[kt][:, 128 * d:128 * (d + 1)],
                            identity=ident)
                        nc.vector.tensor_copy(ttb[:, 128 * kt:128 * (kt + 1)], pb)
                    xaT.append(tta)
                    xbT.append(ttb)

                # kT = Wk.T-contraction: kT[dout, tok] = sum_din Wk[din,dout]*xT[din,tok]
                kaT, kbT = [], []
                for d in range(2):
                    for side, xT, lst in (("a", xaT, kaT), ("b", xbT, kbT)):
                        pk = pss.tile([128, A], F32, name=f"k{side}T_ps",
                                      tag="ps_small")
                        for kt in range(2):
                            nc.tensor.matmul(
                                pk, lhsT=wk_sb[kt][:, 128 * d:128 * (d + 1)],
                                rhs=xT[kt], start=(kt == 0), stop=(kt == 1))
                        ksb = wp.tile([128, A], F32, name=f"k{side}T{d}",
                                      tag=f"k{side}T{d}")
                        nc.vector.tensor_copy(ksb, pk)
                        lst.append(ksb)

                # Per-token L2 norms -> normalized kT (cosine keys).
                kna, knb = [], []
                for side, kT, lst in (("a", kaT, kna), ("b", kbT, knb)):
                    sq = []
                    for d in range(2):
                        sqt = wp.tile([128, A], F32, name=f"sq{side}{d}",
                                      tag=f"kn{side}{d}")
                        nc.vector.tensor_tensor(out=sqt, in0=kT[d], in1=kT[d],
                                                op=AL.mult)
                        sq.append(sqt)
                    pssq = pss.tile([1, A], F32, name=f"ssq{side}", tag="ps_small")
                    for d in range(2):
                        nc.tensor.matmul(pssq, lhsT=onesc, rhs=sq[d],
                                         start=(d == 0), stop=(d == 1))
                    nrm = wp.tile([1, A], F32, name=f"nrm{side}", tag=f"nrm{side}")
                    nc.scalar.sqrt(nrm, pssq)
                    inv = wp.tile([1, A], F32, name=f"inv{side}", tag=f"inv{side}")
                    nc.vector.reciprocal(inv, nrm)
                    pbc = pss.tile([128, A], F32, name=f"bc{side}", tag="ps_small")
                    nc.tensor.matmul(pbc, lhsT=onesr, rhs=inv, start=True,
                                     stop=True)
                    bcs = wp.tile([128, A], F32, name=f"bcs{side}",
                                  tag=f"bcs{side}")
                    nc.vector.tensor_copy(bcs, pbc)
                    for d in range(2):
                        kn = wp.tile([128, A], F32, name=f"kn{side}{d}",
                                     tag=f"kn{side}{d}")
                        nc.vector.tensor_tensor(out=kn, in0=kT[d], in1=bcs,
                                                op=AL.mult)
                        lst.append(kn)

                # scores[i, j] = kna[:, i] . knb[:, j]  -> [256 (2 tiles), 256]
                sc = []
                for m in range(2):
                    psc = pss.tile([128, A], F32, name="sc_ps", tag="ps_small")
                    for d in range(2):
                        nc.tensor.matmul(
                            psc, lhsT=kna[d][:, 128 * m:128 * (m + 1)],
                            rhs=knb[d], start=(d == 0), stop=(d == 1))
                    scs = wp.tile([128, A], F32, name=f"sc{m}", tag=f"sc{m}")
                    nc.vector.tensor_copy(scs, psc)
                    sc.append(scs)

                # Row max (best score) + argmax (first max index = dst).
                best, dst = [], []
                for m in range(2):
                    bst = wp.tile([128, 1], F32, name=f"best{m}", tag=f"best{m}")
                    nc.vector.reduce_max(bst, sc[m], axis=AX.X)
                    am = wp.tile([128, A], F32, name="am", tag="am")
                    nc.vector.scalar_tensor_tensor(
                        out=am, in0=sc[m], scalar=bst[:, :1], in1=iotam,
                        op0=AL.is_ge, op1=AL.mult)
                    dmin = wp.tile([128, 1], F32, name="dmin", tag="dmin")
                    nc.vector.tensor_reduce(dmin, am, axis=AX.X, op=AL.min)
                    dstt = wp.tile([128, 1], F32, name=f"dst{m}", tag=f"dst{m}")
                    nc.vector.tensor_scalar_add(dstt, dmin, BIG)
                    best.append(bst)
                    dst.append(dstt)

                # Broadcast best across partitions: bcb[p, j] = best[j].
                bcb = wp.tile([128, A], F32, name="bcb", tag="bcb")
                for m in range(2):
                    pbb = pss.tile([128, 128], F32, name="bb_ps", tag="ps_small")
                    nc.tensor.transpose(
                        out=pbb, in_=best[m][:, :1].to_broadcast([128, 128]),
                        identity=ident)
                    nc.vector.tensor_copy(bcb[:, 128 * m:128 * (m + 1)], pbb)

                # Dense rank with index tie-break:
                # rank_i = #{j: best_j > best_i} + #{j < i: best_j == best_i}
                msel, unm = [], []
                for m in range(2):
                    eqlt = wp.tile([128, A], F32, name="eqlt", tag="eqlt")
                    nc.vector.scalar_tensor_tensor(
                        out=eqlt, in0=bcb, scalar=best[m][:, :1], in1=ltm[m],
                        op0=AL.is_equal, op1=AL.mult)
                    gt = wp.tile([128, A], F32, name="gt", tag="gt")
                    rank = wp.tile([128, 1], F32, name="rank", tag="rank")
                    nc.vector.scalar_tensor_tensor(
                        out=gt, in0=bcb, scalar=best[m][:, :1], in1=eqlt,
                        op0=AL.is_gt, op1=AL.add, accum_out=rank)
                    ms = wp.tile([128, 1], F32, name=f"msel{m}", tag=f"msel{m}")
                    nc.vector.tensor_scalar(ms, rank, float(R), None, op0=AL.is_lt)
                    un = wp.tile([128, 1], F32, name=f"unm{m}", tag=f"unm{m}")
                    nc.vector.tensor_scalar(un, rank, float(R), None, op0=AL.is_ge)
                    msel.append(ms)
                    unm.append(un)

                # Compaction positions (prefix counts) via strictly-upper matmul,
                # for the unmerged set and the selected set.
                pos, selpos = [], []
                for m in range(2):
                    pps = pss.tile([128, 1], F32, name="pos_ps", tag="ps_small")
                    for kt in range(2):
                        nc.tensor.matmul(pps,
                                         lhsT=sut[kt][:, 128 * m:128 * (m + 1)],
                                         rhs=unm[kt], start=(kt == 0),
                                         stop=(kt == 1))
                    pst = wp.tile([128, 1], F32, name=f"pos{m}", tag=f"pos{m}")
                    nc.vector.tensor_copy(pst, pps)
                    pos.append(pst)
                    sps = pss.tile([128, 1], F32, name="spos_ps", tag="ps_small")
                    for kt in range(2):
                        nc.tensor.matmul(sps,
                                         lhsT=sut[kt][:, 128 * m:128 * (m + 1)],
                                         rhs=msel[kt], start=(kt == 0),
                                         stop=(kt == 1))
                    spt = wp.tile([128, 1], F32, name=f"selpos{m}",
                                  tag=f"selpos{m}")
                    nc.vector.tensor_copy(spt, sps)
                    selpos.append(spt)

                # One-hot compaction matrices (K-side layout).
                punmT, selT = [], []
                for kt in range(2):
                    pu = wp.tile([128, R], F32, name=f"punmT{kt}",
                                 tag=f"punmT{kt}")
                    nc.vector.tensor_scalar(pu, iota[:, :R], pos[kt][:, :1],
                                            unm[kt][:, :1], op0=AL.is_equal,
                                            op1=AL.mult)
                    punmT.append(pu)
                    st = wp.tile([128, R], F32, name=f"selT{kt}",
                                 tag=f"selT{kt}")
                    nc.vector.tensor_scalar(st, iota[:, :R], selpos[kt][:, :1],
                                            msel[kt][:, :1], op0=AL.is_equal,
                                            op1=AL.mult)
                    selT.append(st)

                # DRAM row-index vectors (int32) for the indirect gathers.
                rows_i = []
                for mats, nmv in ((punmT, "unmrows"), (selT, "srcrows")):
                    pix = pss.tile([128, 1], F32, name="pix", tag="ps_small")
                    for kt in range(2):
                        nc.tensor.matmul(pix, lhsT=mats[kt], rhs=iotap[kt],
                                         start=(kt == 0), stop=(kt == 1))
                    rf = wp.tile([128, 1], F32, name=f"{nmv}_f", tag=f"{nmv}_f")
                    # DRAM row of a-token i is r0 + 2i
                    nc.vector.tensor_scalar(rf, pix, 2.0, float(r0),
                                            op0=AL.mult, op1=AL.add)
                    ri = wp.tile([128, 1], I32, name=f"{nmv}_i", tag=f"{nmv}_i")
                    nc.vector.tensor_copy(ri, rf)
                    rows_i.append(ri)
                unmrows, srcrows = rows_i

                # dst per selected row (in compact order) -> one-hot MselT.
                pds = pss.tile([128, 1], F32, name="pds", tag="ps_small")
                for kt in range(2):
                    nc.tensor.matmul(pds, lhsT=selT[kt], rhs=dst[kt],
                                     start=(kt == 0), stop=(kt == 1))
                dstsel = wp.tile([128, 1], F32, name="dstsel", tag="dstsel")
                nc.vector.tensor_copy(dstsel, pds)
                mselT = wp.tile([128, A], F32, name="mselT", tag="mselT")
                nc.vector.tensor_scalar(mselT, iota[:, :A], dstsel[:, :1], None,
                                        op0=AL.is_equal)

                win[w] = dict(xa=xa, xb=xb, mselT=mselT, punmT=punmT,
                              selT=selT, unmrows=unmrows, srcrows=srcrows)

            for w in range(WPC):
                r0 = W * w
                o0 = OUT_W * w
                st = win[w]
                xa, xb = st["xa"], st["xb"]
                mselT, punmT, selT = st["mselT"], st["punmT"], st["selT"]
                unmrows, srcrows = st["unmrows"], st["srcrows"]

                # ================= stage B: stream s =================
                bsnp = []
                for m in range(2):
                    bsnp.append(wp.tile([128, (CHUNK // 512) * NCH], F32, name=f"bsnp{m}",
                                        tag=f"bsnp{m}"))
                ssp = wp.tile([128, NCH], F32, name="ssp", tag="ssp")

                for c in range(NCH):
                    c0 = CHUNK * c
                    unm_t = sp.tile([128, CHUNK], F32, name="unm_t", tag="unm_t", bufs=2)
                    src_t = sp.tile([128, CHUNK], F32, name="src_t", tag="src_t", bufs=3)
                    if DBG_INDIRECT:
                        nc.gpsimd.indirect_dma_start(
                            out=unm_t, out_offset=None, in_=s[:, :],
                            in_offset=bass.IndirectOffsetOnAxis(ap=unmrows[:, :1],
                                                                axis=0),
                            element_offset=c0)
                        nc.gpsimd.indirect_dma_start(
                            out=src_t, out_offset=None, in_=s[:, :],
                            in_offset=bass.IndirectOffsetOnAxis(ap=srcrows[:, :1],
                                                                axis=0),
                            element_offset=c0)
                    else:
                        nc.sync.dma_start(out=unm_t,
                                          in_=s[r0:r0 + 256:2, c0:c0 + CHUNK])
                        nc.sync.dma_start(out=src_t,
                                          in_=s[r0 + 1:r0 + 256:2, c0:c0 + CHUNK])
                    # unmerged rows go straight out
                    _oeng = nc.scalar if DBG_OUT_ENGINE == "scalar" else nc.sync
                    _oeng.dma_start(out=out_s[o0:o0 + R, c0:c0 + CHUNK],
                                    in_=unm_t)
                    # selected-row sizes (row-sum partials)
                    nc.vector.reduce_sum(ssp[:, c:c + 1], src_t, axis=AX.X)

                    bs_t = []
                    for m in range(2):
                        rb = r0 + 256 * m
                        tb = sp.tile([128, CHUNK], F32, name=f"bst{m}",
                                     tag=f"bst{m}", bufs=8)
                        nc.sync.dma_start(out=tb,
                                          in_=s[rb + 1:rb + 256:2, c0:c0 + CHUNK])
                        bs_t.append(tb)

                    for m in range(2):
                        outb = sp.tile([128, CHUNK], F32, name=f"outb{m}",
                                       tag=f"outb{m}", bufs=3)
                        for h in range(CHUNK // 512):
                            hs = 512 * h
                            pB = psb.tile([128, 512], F32, name="pB", tag="pB")
                            nc.tensor.matmul(
                                pB, lhsT=mselT[:, 128 * m:128 * (m + 1)],
                                rhs=src_t[:, hs:hs + 512], start=True, stop=True)
                            if DBG_TTR:
                                # fused (pB + 0) + bs with row-sum accumulator;
                                # NOTE tensor_tensor_reduce hard-crashes TRN2
                                # here — scalar_tensor_tensor is the working
                                # fused form.
                                nc.vector.scalar_tensor_tensor(
                                    out=outb[:, hs:hs + 512], in0=pB,
                                    scalar=0.0, in1=bs_t[m][:, hs:hs + 512],
                                    op0=AL.add, op1=AL.add,
                                    accum_out=bsnp[m][:, (CHUNK // 512) * c + h:(CHUNK // 512) * c + h + 1])
                            else:
                                nc.vector.tensor_tensor(
                                    out=outb[:, hs:hs + 512], in0=pB,
                                    in1=bs_t[m][:, hs:hs + 512], op=AL.add)
                                nc.vector.reduce_sum(
                                    bsnp[m][:, (CHUNK // 512) * c + h:
                                            (CHUNK // 512) * c + h + 1],
                                    outb[:, hs:hs + 512], axis=AX.X)
                        ro = o0 + R + 128 * m
                        _oeng.dma_start(out=out_s[ro:ro + 128, c0:c0 + CHUNK],
                                        in_=outb)

                # ================= stage C: x merge =================
                srcsize = wp.tile([128, 1], F32, name="srcsize", tag="srcsize")
                nc.vector.reduce_sum(srcsize, ssp, axis=AX.X)
                bsnew = []
                for m in range(2):
                    bn = wp.tile([128, 1], F32, name=f"bsnew{m}", tag=f"bsnew{m}")
                    nc.vector.reduce_sum(bn, bsnp[m], axis=AX.X)
                    bsnew.append(bn)

                # selected x rows (compact order), weighted by size
                psx = pss.tile([128, D], F32, name="psx", tag="ps_small")
                for kt in range(2):
                    nc.tensor.matmul(psx, lhsT=selT[kt], rhs=xa[kt],
                                     start=(kt == 0), stop=(kt == 1))
                wsrc = wp.tile([128, D + 1], F32, name="wsrc", tag="wsrc")
                nc.scalar.mul(wsrc[:, :D], psx, srcsize[:, :1])
                nc.vector.tensor_copy(wsrc[:, D:D + 1], srcsize)

                # unmerged-a rows of out_x: plain gather via punmT
                pxa = pss.tile([128, D], F32, name="pxa", tag="ps_small")
                for kt in range(2):
                    nc.tensor.matmul(pxa, lhsT=punmT[kt], rhs=xa[kt],
                                     start=(kt == 0), stop=(kt == 1))
                outxa = wp.tile([128, D], F32, name="outxa", tag="outxa")
                nc.scalar.copy(outxa, pxa)
                (nc.scalar if DBG_OUT_ENGINE == "scalar" else nc.sync).dma_start(
                    out=out_x[o0:o0 + R, :], in_=outxa)

                # merged-b rows: (xb*b_size + Msel @ (srcx*srcsize)) / b_size_new
                for m in range(2):
                    pXB = psb.tile([128, D + 1], F32, name="pB", tag="pB")
                    nc.tensor.matmul(pXB,
                                     lhsT=mselT[:, 128 * m:128 * (m + 1)],
                                     rhs=wsrc, start=True, stop=True)
                    # b_size = b_size_new - (scattered src sizes)
                    bsz = wp.tile([128, 1], F32, name="bsz", tag="bsz")
                    nc.vector.tensor_tensor(out=bsz, in0=bsnew[m],
                                            in1=pXB[:, D:D + 1], op=AL.subtract)
                    wxb = wp.tile([128, D], F32, name="wxb", tag="wxb")
                    nc.scalar.mul(wxb, xb[m], bsz[:, :1])
                    numb = wp.tile([128, D], F32, name="numb", tag="numb")
                    nc.vector.tensor_tensor(out=numb, in0=pXB[:, :D], in1=wxb,
                                            op=AL.add)
                    den = wp.tile([128, 1], F32, name="den", tag="den")
                    nc.vector.tensor_scalar_max(den, bsnew[m], EPS)
                    rec = wp.tile([128, 1], F32, name="rec", tag="rec")
                    nc.vector.reciprocal(rec, den)
                    outxb = wp.tile([128, D], F32, name="outxb", tag="outxb")
                    nc.scalar.mul(outxb, numb, rec[:, :1])
                    ro = o0 + R + 128 * m
                    (nc.scalar if DBG_OUT_ENGINE == "scalar" else nc.sync).dma_start(
                        out=out_x[ro:ro + 128, :], in_=outxb)

    nc.compile()
    return nc


def host_constants() -> dict[str, np.ndarray]:
    i = np.arange(A, dtype=np.float32)
    return {
        "ident": np.eye(128, dtype=np.float32),
        "iota512": np.broadcast_to(np.arange(512, dtype=np.float32),
                                   (128, 512)).copy(),
        "iotam": np.broadcast_to(np.arange(256, dtype=np.float32) - BIG,
                                 (128, 256)).copy(),
        "iotap": i[:, None].copy(),
        # sut[j, i] = 1 if j < i  (strictly upper): prefix-count lhsT
        "sut": (i[:, None] < i[None, :]).astype(np.float32),
        # ltm[i, j] = 1 if j < i  (strictly lower): tie-break mask
        "ltm": (i[None, :] < i[:, None]).astype(np.float32),
        "ones_col": np.ones((128, 1), dtype=np.float32),
        "ones_row": np.ones((1, 128), dtype=np.float32),
    }


_CACHED_NC = None


def kernel(x: np.ndarray, s: np.ndarray, Wk: np.ndarray):
    from concourse.bass_utils import run_bass_kernel_spmd

    global _CACHED_NC
    if _CACHED_NC is None:
        _CACHED_NC = build_program()
    nc = _CACHED_NC

    x = np.ascontiguousarray(np.asarray(x, dtype=np.float32))
    s = np.ascontiguousarray(np.asarray(s, dtype=np.float32))
    Wk = np.ascontiguousarray(np.asarray(Wk, dtype=np.float32))
    consts = host_constants()

    in_maps = []
    for c in range(NCORES):
        r = ROWS_PC * c
        in_maps.append({
            "x": x[r:r + ROWS_PC],
            "s": s[r:r + ROWS_PC],
            "wk": Wk,
            **consts,
        })

    res = run_bass_kernel_spmd(nc, in_maps, list(range(NCORES)))
    new_x = np.concatenate([res.results[c]["out_x"] for c in range(NCORES)],
                           axis=0)
    new_s = np.concatenate([res.results[c]["out_s"] for c in range(NCORES)],
                           axis=0)
    return new_x, new_s


# revision 19
# speedup vs baseline: 1.2615x; 1.0260x over previous
"""Trainium2 Bass kernel for the MergeDNA window-local ToMe merge.

Problem (T=8192 tokens, D=256, N=8192, WINDOW=512, N_SELECT=128):
  k = l2norm(x @ Wk); per 512-token window: bipartite-match even (a) tokens
  to odd (b) tokens by cosine score, merge the top-128 a-tokens into their
  best b-match (size-weighted for x, plain sum for s), emit
  [unmerged-a (128 rows, order preserved); merged-b (256 rows)] per window.

Strategy (per core, sequence-parallel over windows, 2 windows/core):
  Stage A turns all data-dependent control into tiny on-chip artifacts:
    - dense rank of best-scores (compare matrix + row-reduce) -> top-128 mask
    - prefix-sum matmuls -> compaction positions for unmerged/selected rows
    - DRAM row-index vectors (int32) for the unmerged and selected a-rows
    - MselT [128,256]: one-hot dest b-row per selected a-row
  Stage B streams the [512, 8192] s-window once:
    - unmerged a-rows: indirect-DMA row gather -> straight DMA to out_s
      (zero compute)
    - selected a-rows: indirect-DMA row gather -> one K=128 matmul per
      [128,512] output tile scatters them onto b rows; a fused DVE
      scalar_tensor_tensor adds streamed b_s and accumulates row sums
  Stage C reuses the same one-hot matrices on [128,257] tiles for the
  size-weighted x merge.

The kernel function takes FULL inputs, shards rows across 8 NeuronCores
(windows are independent), and concatenates the per-core outputs.
"""

import numpy as np

import concourse.bass as bass
import concourse.mybir as mybir
from concourse import bacc
from concourse.tile import TileContext

F32 = mybir.dt.float32
I32 = mybir.dt.int32

T, D, NS = 8192, 256, 8192
W = 512                     # window size (tokens)
A = W // 2                  # tokens per side (a=even, b=odd) = 256
R = 128                     # N_SELECT: merged tokens per window
OUT_W = W - R               # output rows per window = 384
NW = T // W                 # 16 windows
NCORES = 8
WPC = NW // NCORES          # 2 windows per core
ROWS_PC = W * WPC           # 1024 input rows per core
OUT_PC = OUT_W * WPC        # 768 output rows per core
CHUNK = 1024                # columns of s per streamed chunk
NCH = NS // CHUNK           # 8 chunks
HPC = CHUNK // 512          # 512-wide matmul slices per chunk
EPS = 1e-6
BIG = 65536.0

# packed-constant column offsets
C_ID, C_IOTA, C_IOTAM = 0, 128, 640
C_SUT, C_LTM, C_WK = 896, 1408, 1920
C_IOTAP, C_ONES = 2432, 2434
C_TOT = C_ONES + 128


def build_program() -> bass.Bass:
    nc = bacc.Bacc("TRN2")

    x = nc.dram_tensor("x", [ROWS_PC, D], F32, kind="ExternalInput")
    s = nc.dram_tensor("s", [ROWS_PC, NS], F32, kind="ExternalInput")
    cpack_d = nc.dram_tensor("cpack", [128, C_TOT], F32, kind="ExternalInput")

    out_x = nc.dram_tensor("out_x", [OUT_PC, D], F32, kind="ExternalOutput")
    out_s = nc.dram_tensor("out_s", [OUT_PC, NS], F32, kind="ExternalOutput")

    AL = mybir.AluOpType
    AX = mybir.AxisListType

    with TileContext(nc) as tc:
        with (
            tc.tile_pool(name="const", bufs=1) as cp,
            tc.tile_pool(name="work", bufs=2) as wp,
            tc.tile_pool(name="stream", bufs=4) as sp,
            tc.tile_pool(name="pss", bufs=4, space="PSUM") as pss,
            tc.tile_pool(name="psb", bufs=4, space="PSUM") as psb,
        ):
            # ---- stage-A inputs first: x rows on the sync queue, the one
            # packed constant tensor in parallel on the scalar queue ----
            win = {}
            for w in range(WPC):
                r0 = W * w
                xa, xb = [], []
                for kt in range(2):
                    rb = r0 + 256 * kt
                    ta = wp.tile([128, D], F32, name=f"xa{kt}", tag=f"xa{kt}")
                    nc.sync.dma_start(out=ta, in_=x[rb:rb + 256:2, :])
                    xa.append(ta)
                    tb = wp.tile([128, D], F32, name=f"xb{kt}", tag=f"xb{kt}")
                    nc.sync.dma_start(out=tb, in_=x[rb + 1:rb + 256:2, :])
                    xb.append(tb)
                win[w] = dict(xa=xa, xb=xb)

            cpk = cp.tile([128, C_TOT], F32, name="cpk", tag="cpk")
            nc.scalar.dma_start(out=cpk, in_=cpack_d[:, :])
            ident = cpk[:, C_ID:C_ID + 128]
            iota = cpk[:, C_IOTA:C_IOTA + 512]
            iotam = cpk[:, C_IOTAM:C_IOTAM + 256]
            sut = [cpk[:, C_SUT + 256 * kt:C_SUT + 256 * (kt + 1)]
                   for kt in range(2)]
            ltm = [cpk[:, C_LTM + 256 * kt:C_LTM + 256 * (kt + 1)]
                   for kt in range(2)]
            wk_sb = [cpk[:, C_WK + 256 * kt:C_WK + 256 * (kt + 1)]
                     for kt in range(2)]
            iotap = [cpk[:, C_IOTAP + kt:C_IOTAP + kt + 1] for kt in range(2)]
            onesc = cpk[:, C_ONES:C_ONES + 1]
            onesr = cpk[0:1, C_ONES:C_ONES + 128]

            for w in range(WPC):
                r0 = W * w          # input row base of this window

                # ================= stage A: selection =================
                xa, xb = win[w]["xa"], win[w]["xb"]

                # Transpose x to [din, token] layout (PE transpose by blocks).
                xaT, xbT = [], []
                for d in range(2):
                    tta = wp.tile([128, A], F32, name=f"xaT{d}", tag=f"xaT{d}")
                    ttb = wp.tile([128, A], F32, name=f"xbT{d}", tag=f"xbT{d}")
                    for kt in range(2):
                        pa = pss.tile([128, 128], F32, name="tp_a", tag="ps_small")
                        nc.tensor.transpose(
                            out=pa, in_=xa[kt][:, 128 * d:128 * (d + 1)],
                            identity=ident)
                        nc.vector.tensor_copy(tta[:, 128 * kt:128 * (kt + 1)], pa)
                        pb = pss.tile([128, 128], F32, name="tp_b", tag="ps_small")
                        nc.tensor.transpose(
                            out=pb, in_=xb[kt][:, 128 * d:128 * (d + 1)],
                            identity=ident)
                        nc.vector.tensor_copy(ttb[:, 128 * kt:128 * (kt + 1)], pb)
                    xaT.append(tta)
                    xbT.append(ttb)

                # kT[dout, tok] = sum_din Wk[din,dout] * xT[din,tok]
                kaT, kbT = [], []
                for d in range(2):
                    for side, xT, lst in (("a", xaT, kaT), ("b", xbT, kbT)):
                        pk = pss.tile([128, A], F32, name=f"k{side}T_ps",
                                      tag="ps_small")
                        for kt in range(2):
                            nc.tensor.matmul(
                                pk, lhsT=wk_sb[kt][:, 128 * d:128 * (d + 1)],
                                rhs=xT[kt], start=(kt == 0), stop=(kt == 1))
                        ksb = wp.tile([128, A], F32, name=f"k{side}T{d}",
                                      tag=f"k{side}T{d}")
                        nc.vector.tensor_copy(ksb, pk)
                        lst.append(ksb)

                # Per-token L2 norms -> normalized kT (cosine keys).
                # Phase-split so the a/b sub-chains interleave on each engine;
                # reciprocal runs wide ([128, A]) after the broadcast.
                sqs, ssqs, nrms, bcps, invs = {}, {}, {}, {}, {}
                for side, kT in (("a", kaT), ("b", kbT)):
                    for d in range(2):
                        sqt = wp.tile([128, A], F32, name=f"sq{side}{d}",
                                      tag=f"kn{side}{d}")
                        nc.vector.tensor_tensor(out=sqt, in0=kT[d], in1=kT[d],
                                                op=AL.mult)
                        sqs[side, d] = sqt
                for side in ("a", "b"):
                    pssq = pss.tile([1, A], F32, name=f"ssq{side}",
                                    tag="ps_small")
                    for d in range(2):
                        nc.tensor.matmul(pssq, lhsT=onesc, rhs=sqs[side, d],
                                         start=(d == 0), stop=(d == 1))
                    ssqs[side] = pssq
                for side in ("a", "b"):
                    nrm = wp.tile([1, A], F32, name=f"nrm{side}",
                                  tag=f"nrm{side}")
                    nc.scalar.sqrt(nrm, ssqs[side])
                    nrms[side] = nrm
                for side in ("a", "b"):
                    pbc = pss.tile([128, A], F32, name=f"bc{side}",
                                   tag="ps_small")
                    nc.tensor.matmul(pbc, lhsT=onesr, rhs=nrms[side],
                                     start=True, stop=True)
                    bcps[side] = pbc
                for side in ("a", "b"):
                    inv = wp.tile([128, A], F32, name=f"invb{side}",
                                  tag=f"invb{side}")
                    nc.vector.reciprocal(inv, bcps[side])
                    invs[side] = inv
                kna, knb = [], []
                for side, kT, lst in (("a", kaT, kna), ("b", kbT, knb)):
                    for d in range(2):
                        kn = wp.tile([128, A], F32, name=f"kn{side}{d}",
                                     tag=f"kn{side}{d}")
                        nc.vector.tensor_tensor(out=kn, in0=kT[d],
                                                in1=invs[side], op=AL.mult)
                        lst.append(kn)

                # scores[i, j] = kna[:, i] . knb[:, j]  -> [256 (2 tiles), 256]
                sc = []
                for m in range(2):
                    psc = pss.tile([128, A], F32, name="sc_ps", tag="ps_small")
                    for d in range(2):
                        nc.tensor.matmul(
                            psc, lhsT=kna[d][:, 128 * m:128 * (m + 1)],
                            rhs=knb[d], start=(d == 0), stop=(d == 1))
                    scs = wp.tile([128, A], F32, name=f"sc{m}", tag=f"sc{m}")
                    nc.vector.tensor_copy(scs, psc)
                    sc.append(scs)

                # Row max (best score) + argmax (first max index = dst).
                # dst lands in column 1 of pixrhs (iotap const in column 0)
                # so one [128,2] matmul later yields src-rows AND dstsel.
                best, dst, pixrhs = [], [], []
                for m in range(2):
                    bst = wp.tile([128, 1], F32, name=f"best{m}", tag=f"best{m}")
                    nc.vector.reduce_max(bst, sc[m], axis=AX.X)
                    best.append(bst)
                    pr = wp.tile([128, 2], F32, name=f"pixr{m}", tag=f"pixr{m}")
                    nc.vector.tensor_copy(pr[:, 0:1], iotap[m])
                    pixrhs.append(pr)
                for m in range(2):
                    am = wp.tile([128, A], F32, name="am", tag="am")
                    nc.vector.scalar_tensor_tensor(
                        out=am, in0=sc[m], scalar=best[m][:, :1], in1=iotam,
                        op0=AL.is_ge, op1=AL.mult)
                    dmin = wp.tile([128, 1], F32, name="dmin", tag="dmin")
                    nc.vector.tensor_reduce(dmin, am, axis=AX.X, op=AL.min)
                    dstt = wp.tile([128, 1], F32, name=f"dst{m}", tag=f"dst{m}")
                    nc.vector.tensor_scalar_add(dstt, dmin, BIG)
                    dst.append(dstt)

                # Broadcast best across partitions: bcb[p, j] = best[j].
                bcb = wp.tile([128, A], F32, name="bcb", tag="bcb")
                for m in range(2):
                    pbb = pss.tile([128, 128], F32, name="bb_ps", tag="ps_small")
                    nc.tensor.transpose(
                        out=pbb, in_=best[m][:, :1].to_broadcast([128, 128]),
                        identity=ident)
                    nc.vector.tensor_copy(bcb[:, 128 * m:128 * (m + 1)], pbb)

                # Dense rank with index tie-break:
                # rank_i = #{j: best_j > best_i} + #{j < i: best_j == best_i}
                # msk[:, 0] = selected (rank < R), msk[:, 1] = unmerged.
                msk = []
                for m in range(2):
                    eqlt = wp.tile([128, A], F32, name="eqlt", tag="eqlt")
                    nc.vector.scalar_tensor_tensor(
                        out=eqlt, in0=bcb, scalar=best[m][:, :1], in1=ltm[m],
                        op0=AL.is_equal, op1=AL.mult)
                    gt = wp.tile([128, A], F32, name="gt", tag="gt")
                    rank = wp.tile([128, 1], F32, name="rank", tag=f"rank{m}")
                    nc.vector.scalar_tensor_tensor(
                        out=gt, in0=bcb, scalar=best[m][:, :1], in1=eqlt,
                        op0=AL.is_gt, op1=AL.add, accum_out=rank)
                    mk = wp.tile([128, 2], F32, name=f"msk{m}", tag=f"msk{m}")
                    nc.vector.tensor_scalar(mk[:, 0:1], rank, float(R), None,
                                            op0=AL.is_lt)
                    nc.vector.tensor_scalar(mk[:, 1:2], rank, float(R), None,
                                            op0=AL.is_ge)
                    msk.append(mk)

                # Compaction prefix counts for both sets in one N=2 matmul:
                # col 0 = position among selected, col 1 = among unmerged.
                pos2 = []
                for m in range(2):
                    pps = pss.tile([128, 2], F32, name="pos_ps", tag="ps_small")
                    for kt in range(2):
                        nc.tensor.matmul(pps,
                                         lhsT=sut[kt][:, 128 * m:128 * (m + 1)],
                                         rhs=msk[kt], start=(kt == 0),
                                         stop=(kt == 1))
                    pst = wp.tile([128, 2], F32, name=f"pos2{m}", tag=f"pos2{m}")
                    nc.vector.tensor_copy(pst, pps)
                    pos2.append(pst)

                # One-hot compaction matrices (K-side layout).
                punmT, selT = [], []
                for kt in range(2):
                    pu = wp.tile([128, R], F32, name=f"punmT{kt}",
                                 tag=f"punmT{kt}")
                    nc.vector.tensor_scalar(pu, iota[:, :R],
                                            pos2[kt][:, 1:2],
                                            msk[kt][:, 1:2], op0=AL.is_equal,
                                            op1=AL.mult)
                    punmT.append(pu)
                    st = wp.tile([128, R], F32, name=f"selT{kt}",
                                 tag=f"selT{kt}")
                    nc.vector.tensor_scalar(st, iota[:, :R],
                                            pos2[kt][:, 0:1],
                                            msk[kt][:, 0:1], op0=AL.is_equal,
                                            op1=AL.mult)
                    selT.append(st)

                # Row indices + dstsel via two batched matmuls:
                #   selT.T @ [iotap | dst] -> [src a-idx | dstsel]
                #   punmT.T @ iotap       -> unm a-idx
                for m in range(2):
                    nc.vector.tensor_copy(pixrhs[m][:, 1:2], dst[m])
                psel = pss.tile([128, 2], F32, name="psel", tag="ps_small")
                for kt in range(2):
                    nc.tensor.matmul(psel, lhsT=selT[kt], rhs=pixrhs[kt],
                                     start=(kt == 0), stop=(kt == 1))
                punm = pss.tile([128, 1], F32, name="punm", tag="ps_small")
                for kt in range(2):
                    nc.tensor.matmul(punm, lhsT=punmT[kt],
                                     rhs=pixrhs[kt][:, 0:1],
                                     start=(kt == 0), stop=(kt == 1))

                srf = wp.tile([128, 1], F32, name="srf", tag="srf")
                nc.vector.tensor_scalar(srf, psel[:, 0:1], 2.0, float(r0),
                                        op0=AL.mult, op1=AL.add)
                srcrows = wp.tile([128, 1], I32, name="srcrows", tag="srcrows")
                nc.vector.tensor_copy(srcrows, srf)
                dstsel = wp.tile([128, 1], F32, name="dstsel", tag="dstsel")
                nc.vector.tensor_copy(dstsel, psel[:, 1:2])
                urf = wp.tile([128, 1], F32, name="urf", tag="urf")
                nc.vector.tensor_scalar(urf, punm, 2.0, float(r0),
                                        op0=AL.mult, op1=AL.add)
                unmrows = wp.tile([128, 1], I32, name="unmrows", tag="unmrows")
                nc.vector.tensor_copy(unmrows, urf)
                mselT = wp.tile([128, A], F32, name="mselT", tag="mselT")
                nc.vector.tensor_scalar(mselT, iota[:, :A], dstsel[:, :1], None,
                                        op0=AL.is_equal)

                win[w].update(mselT=mselT, punmT=punmT, selT=selT,
                              unmrows=unmrows, srcrows=srcrows)

            for w in range(WPC):
                r0 = W * w
                o0 = OUT_W * w
                st = win[w]
                xa, xb = st["xa"], st["xb"]
                mselT, punmT, selT = st["mselT"], st["punmT"], st["selT"]
                unmrows, srcrows = st["unmrows"], st["srcrows"]

                # ================= stage B: stream s =================
                bsnp = []
                for m in range(2):
                    bsnp.append(wp.tile([128, HPC * NCH], F32, name=f"bsnp{m}",
                                        tag=f"bsnp{m}"))
                ssp = wp.tile([128, NCH], F32, name="ssp", tag="ssp")

                for c in range(NCH):
                    c0 = CHUNK * c
                    unm_t = sp.tile([128, CHUNK], F32, name="unm_t",
                                    tag="unm_t", bufs=2)
                    nc.gpsimd.indirect_dma_start(
                        out=unm_t, out_offset=None, in_=s[:, :],
                        in_offset=bass.IndirectOffsetOnAxis(ap=unmrows[:, :1],
                                                            axis=0),
                        element_offset=c0)
                    src_t = sp.tile([128, CHUNK], F32, name="src_t",
                                    tag="src_t", bufs=3)
                    nc.gpsimd.indirect_dma_start(
                        out=src_t, out_offset=None, in_=s[:, :],
                        in_offset=bass.IndirectOffsetOnAxis(ap=srcrows[:, :1],
                                                            axis=0),
                        element_offset=c0)
                    # unmerged rows go straight out
                    nc.scalar.dma_start(out=out_s[o0:o0 + R, c0:c0 + CHUNK],
                                        in_=unm_t)
                    # selected-row sizes (row-sum partials)
                    nc.vector.reduce_sum(ssp[:, c:c + 1], src_t, axis=AX.X)

                    bs_t = []
                    for m in range(2):
                        rb = r0 + 256 * m
                        tb = sp.tile([128, CHUNK], F32, name=f"bst{m}",
                                     tag=f"bst{m}", bufs=8)
                        nc.sync.dma_start(out=tb,
                                          in_=s[rb + 1:rb + 256:2, c0:c0 + CHUNK])
                        bs_t.append(tb)

                    for m in range(2):
                        outb = sp.tile([128, CHUNK], F32, name=f"outb{m}",
                                       tag=f"outb{m}", bufs=3)
                        for h in range(HPC):
                            hs = 512 * h
                            pB = psb.tile([128, 512], F32, name="pB", tag="pB")
                            nc.tensor.matmul(
                                pB, lhsT=mselT[:, 128 * m:128 * (m + 1)],
                                rhs=src_t[:, hs:hs + 512], start=True, stop=True)
                            # fused (pB + 0) + bs with row-sum accumulator
                            # (tensor_tensor_reduce hard-crashes TRN2 here;
                            # scalar_tensor_tensor is the working fused form)
                            nc.vector.scalar_tensor_tensor(
                                out=outb[:, hs:hs + 512], in0=pB,
                                scalar=0.0, in1=bs_t[m][:, hs:hs + 512],
                                op0=AL.add, op1=AL.add,
                                accum_out=bsnp[m][:, HPC * c + h:HPC * c + h + 1])
                        ro = o0 + R + 128 * m
                        nc.scalar.dma_start(out=out_s[ro:ro + 128, c0:c0 + CHUNK],
                                            in_=outb)

                # ================= stage C: x merge =================
                srcsize = wp.tile([128, 1], F32, name="srcsize", tag="srcsize")
                nc.vector.reduce_sum(srcsize, ssp, axis=AX.X)
                bsnew = []
                for m in range(2):
                    bn = wp.tile([128, 1], F32, name=f"bsnew{m}", tag=f"bsnew{m}")
                    nc.vector.reduce_sum(bn, bsnp[m], axis=AX.X)
                    bsnew.append(bn)

                # selected x rows (compact order), weighted by size
                psx = pss.tile([128, D], F32, name="psx", tag="ps_small")
                for kt in range(2):
                    nc.tensor.matmul(psx, lhsT=selT[kt], rhs=xa[kt],
                                     start=(kt == 0), stop=(kt == 1))
                wsrc = wp.tile([128, D + 1], F32, name="wsrc", tag="wsrc")
                nc.scalar.mul(wsrc[:, :D], psx, srcsize[:, :1])
                nc.vector.tensor_copy(wsrc[:, D:D + 1], srcsize)

                # unmerged-a rows of out_x: plain gather via punmT
                pxa = pss.tile([128, D], F32, name="pxa", tag="ps_small")
                for kt in range(2):
                    nc.tensor.matmul(pxa, lhsT=punmT[kt], rhs=xa[kt],
                                     start=(kt == 0), stop=(kt == 1))
                outxa = wp.tile([128, D], F32, name="outxa", tag="outxa")
                nc.scalar.copy(outxa, pxa)
                nc.scalar.dma_start(out=out_x[o0:o0 + R, :], in_=outxa)

                # merged-b rows: (xb*b_size + Msel @ (srcx*srcsize)) / b_size_new
                for m in range(2):
                    pXB = psb.tile([128, D + 1], F32, name="pB", tag="pB")
                    nc.tensor.matmul(pXB,
                                     lhsT=mselT[:, 128 * m:128 * (m + 1)],
                                     rhs=wsrc, start=True, stop=True)
                    # b_size = b_size_new - (scattered src sizes)
                    bsz = wp.tile([128, 1], F32, name="bsz", tag="bsz")
                    nc.vector.tensor_tensor(out=bsz, in0=bsnew[m],
                                            in1=pXB[:, D:D + 1], op=AL.subtract)
                    wxb = wp.tile([128, D], F32, name="wxb", tag="wxb")
                    nc.scalar.mul(wxb, xb[m], bsz[:, :1])
                    numb = wp.tile([128, D], F32, name="numb", tag="numb")
                    nc.vector.tensor_tensor(out=numb, in0=pXB[:, :D], in1=wxb,
                                            op=AL.add)
                    den = wp.tile([128, 1], F32, name="den", tag="den")
                    nc.vector.tensor_scalar_max(den, bsnew[m], EPS)
                    rec = wp.tile([128, 1], F32, name="rec", tag="rec")
                    nc.vector.reciprocal(rec, den)
                    outxb = wp.tile([128, D], F32, name="outxb", tag="outxb")
                    nc.scalar.mul(outxb, numb, rec[:, :1])
                    ro = o0 + R + 128 * m
                    nc.scalar.dma_start(out=out_x[ro:ro + 128, :], in_=outxb)

    nc.compile()
    return nc


def host_constants(Wk: np.ndarray) -> dict[str, np.ndarray]:
    i = np.arange(A, dtype=np.float32)
    pk = np.zeros((128, C_TOT), dtype=np.float32)
    pk[:, C_ID:C_ID + 128] = np.eye(128, dtype=np.float32)
    pk[:, C_IOTA:C_IOTA + 512] = np.arange(512, dtype=np.float32)[None, :]
    pk[:, C_IOTAM:C_IOTAM + 256] = (np.arange(256, dtype=np.float32)
                                    - BIG)[None, :]
    sut = (i[:, None] < i[None, :]).astype(np.float32)   # sut[j,i]=1 if j<i
    ltm = (i[None, :] < i[:, None]).astype(np.float32)   # ltm[i,j]=1 if j<i
    pk[:, C_SUT:C_SUT + 256] = sut[0:128]
    pk[:, C_SUT + 256:C_SUT + 512] = sut[128:256]
    pk[:, C_LTM:C_LTM + 256] = ltm[0:128]
    pk[:, C_LTM + 256:C_LTM + 512] = ltm[128:256]
    Wk = np.asarray(Wk, np.float32)
    pk[:, C_WK:C_WK + 256] = Wk[0:128]
    pk[:, C_WK + 256:C_WK + 512] = Wk[128:256]
    pk[:, C_IOTAP] = np.arange(128, dtype=np.float32)
    pk[:, C_IOTAP + 1] = 128.0 + np.arange(128, dtype=np.float32)
    pk[:, C_ONES:C_ONES + 128] = 1.0
    return {"cpack": pk}


_CACHED_NC = None


def kernel(x: np.ndarray, s: np.ndarray, Wk: np.ndarray):
    from concourse.bass_utils import run_bass_kernel_spmd

    global _CACHED_NC
    if _CACHED_NC is None:
        _CACHED_NC = build_program()
    nc = _CACHED_NC

    x = np.ascontiguousarray(np.asarray(x, dtype=np.float32))
    s = np.ascontiguousarray(np.asarray(s, dtype=np.float32))
    consts = host_constants(Wk)

    in_maps = []
    for c in range(NCORES):
        r = ROWS_PC * c
        in_maps.append({
            "x": x[r:r + ROWS_PC],
            "s": s[r:r + ROWS_PC],
            **consts,
        })

    res = run_bass_kernel_spmd(nc, in_maps, list(range(NCORES)))
    new_x = np.concatenate([res.results[c]["out_x"] for c in range(NCORES)],
                           axis=0)
    new_s = np.concatenate([res.results[c]["out_s"] for c in range(NCORES)],
                           axis=0)
    return new_x, new_s


# revision 20
# speedup vs baseline: 1.3176x; 1.0444x over previous
"""Trainium2 Bass kernel for the MergeDNA window-local ToMe merge.

Problem (T=8192 tokens, D=256, N=8192, WINDOW=512, N_SELECT=128):
  k = l2norm(x @ Wk); per 512-token window: bipartite-match even (a) tokens
  to odd (b) tokens by cosine score, merge the top-128 a-tokens into their
  best b-match (size-weighted for x, plain sum for s), emit
  [unmerged-a (128 rows, order preserved); merged-b (256 rows)] per window.

Strategy (per core, sequence-parallel over windows, 2 windows/core):
  Stage A turns all data-dependent control into tiny on-chip artifacts:
    - dense rank of best-scores (compare matrix + row-reduce) -> top-128 mask
    - prefix-sum matmuls -> compaction positions for unmerged/selected rows
    - DRAM row-index vectors (int32) for the unmerged and selected a-rows
    - MselT [128,256]: one-hot dest b-row per selected a-row
  Stage B streams the [512, 8192] s-window once:
    - unmerged a-rows: indirect-DMA row gather -> straight DMA to out_s
      (zero compute)
    - selected a-rows: indirect-DMA row gather -> one K=128 matmul per
      [128,512] output tile scatters them onto b rows; a fused DVE
      scalar_tensor_tensor adds streamed b_s and accumulates row sums
  Stage C reuses the same one-hot matrices on [128,257] tiles for the
  size-weighted x merge.

The kernel function takes FULL inputs, shards rows across 8 NeuronCores
(windows are independent), and concatenates the per-core outputs.
"""

import numpy as np

import concourse.bass as bass
import concourse.mybir as mybir
from concourse import bacc
from concourse.tile import TileContext

F32 = mybir.dt.float32
I32 = mybir.dt.int32

T, D, NS = 8192, 256, 8192
W = 512                     # window size (tokens)
A = W // 2                  # tokens per side (a=even, b=odd) = 256
R = 128                     # N_SELECT: merged tokens per window
OUT_W = W - R               # output rows per window = 384
NW = T // W                 # 16 windows
NCORES = 8
WPC = NW // NCORES          # 2 windows per core
ROWS_PC = W * WPC           # 1024 input rows per core
OUT_PC = OUT_W * WPC        # 768 output rows per core
CHUNK = 1024                # columns of s per streamed chunk
NCH = NS // CHUNK           # 8 chunks
HPC = CHUNK // 512          # 512-wide matmul slices per chunk
EPS = 1e-6
BIG = 65536.0

# packed-constant column offsets
C_ID, C_IOTA, C_IOTAM = 0, 128, 640
C_SUT, C_LTM, C_WK = 896, 1408, 1920
C_IOTAP, C_ONES = 2432, 2434
C_TOT = C_ONES + 128


def build_program() -> bass.Bass:
    nc = bacc.Bacc("TRN2")

    x = nc.dram_tensor("x", [ROWS_PC, D], F32, kind="ExternalInput")
    s = nc.dram_tensor("s", [ROWS_PC, NS], F32, kind="ExternalInput")
    xt = nc.dram_tensor("xt", [D, ROWS_PC], F32, kind="ExternalInput")
    cpack_d = nc.dram_tensor("cpack", [128, C_TOT], F32, kind="ExternalInput")

    out_x = nc.dram_tensor("out_x", [OUT_PC, D], F32, kind="ExternalOutput")
    out_s = nc.dram_tensor("out_s", [OUT_PC, NS], F32, kind="ExternalOutput")

    AL = mybir.AluOpType
    AX = mybir.AxisListType

    with TileContext(nc) as tc:
        with (
            tc.tile_pool(name="const", bufs=1) as cp,
            tc.tile_pool(name="work", bufs=2) as wp,
            tc.tile_pool(name="stream", bufs=4) as sp,
            tc.tile_pool(name="pss", bufs=4, space="PSUM") as pss,
            tc.tile_pool(name="psb", bufs=4, space="PSUM") as psb,
        ):
            # ---- stage-A inputs first: x rows on the sync queue, the one
            # packed constant tensor in parallel on the scalar queue ----
            win = {}
            for w in range(WPC):
                r0 = W * w
                xa, xb = [], []
                for kt in range(2):
                    rb = r0 + 256 * kt
                    ta = wp.tile([128, D], F32, name=f"xa{kt}", tag=f"xa{kt}")
                    nc.sync.dma_start(out=ta, in_=x[rb:rb + 256:2, :])
                    xa.append(ta)
                    tb = wp.tile([128, D], F32, name=f"xb{kt}", tag=f"xb{kt}")
                    nc.sync.dma_start(out=tb, in_=x[rb + 1:rb + 256:2, :])
                    xb.append(tb)
                win[w] = dict(xa=xa, xb=xb)

            xt_sb = []
            for kt in range(2):
                tx = cp.tile([128, ROWS_PC], F32, name=f"xt{kt}", tag=f"xt{kt}")
                nc.sync.dma_start(out=tx, in_=xt[128 * kt:128 * (kt + 1), :])
                xt_sb.append(tx)

            cpk = cp.tile([128, C_TOT], F32, name="cpk", tag="cpk")
            nc.scalar.dma_start(out=cpk, in_=cpack_d[:, :])
            ident = cpk[:, C_ID:C_ID + 128]
            iota = cpk[:, C_IOTA:C_IOTA + 512]
            iotam = cpk[:, C_IOTAM:C_IOTAM + 256]
            sut = [cpk[:, C_SUT + 256 * kt:C_SUT + 256 * (kt + 1)]
                   for kt in range(2)]
            ltm = [cpk[:, C_LTM + 256 * kt:C_LTM + 256 * (kt + 1)]
                   for kt in range(2)]
            wk_sb = [cpk[:, C_WK + 256 * kt:C_WK + 256 * (kt + 1)]
                     for kt in range(2)]
            iotap = [cpk[:, C_IOTAP + kt:C_IOTAP + kt + 1] for kt in range(2)]
            onesc = cpk[:, C_ONES:C_ONES + 1]
            onesr = cpk[0:1, C_ONES:C_ONES + 128]

            for w in range(WPC):
                r0 = W * w          # input row base of this window

                # ================= stage A: selection =================
                xa, xb = win[w]["xa"], win[w]["xb"]

                # kT[dout, tok] = sum_din Wk[din,dout] * xT[din,tok]
                # (xT comes host-transposed; a/b = even/odd token columns)
                kaT, kbT = [], []
                for d in range(2):
                    for side, off, lst in (("a", 0, kaT), ("b", 1, kbT)):
                        pk = pss.tile([128, A], F32, name=f"k{side}T_ps",
                                      tag="ps_small")
                        for kt in range(2):
                            nc.tensor.matmul(
                                pk, lhsT=wk_sb[kt][:, 128 * d:128 * (d + 1)],
                                rhs=xt_sb[kt][:, W * w + off:W * (w + 1):2],
                                start=(kt == 0), stop=(kt == 1))
                        ksb = wp.tile([128, A], F32, name=f"k{side}T{d}",
                                      tag=f"k{side}T{d}")
                        nc.vector.tensor_copy(ksb, pk)
                        lst.append(ksb)

                # Per-token L2 norms -> normalized kT (cosine keys).
                # Phase-split so the a/b sub-chains interleave on each engine;
                # reciprocal runs wide ([128, A]) after the broadcast.
                sqs, ssqs, nrms, bcps, invs = {}, {}, {}, {}, {}
                for side, kT in (("a", kaT), ("b", kbT)):
                    for d in range(2):
                        sqt = wp.tile([128, A], F32, name=f"sq{side}{d}",
                                      tag=f"kn{side}{d}")
                        nc.vector.tensor_tensor(out=sqt, in0=kT[d], in1=kT[d],
                                                op=AL.mult)
                        sqs[side, d] = sqt
                for side in ("a", "b"):
                    pssq = pss.tile([1, A], F32, name=f"ssq{side}",
                                    tag="ps_small")
                    for d in range(2):
                        nc.tensor.matmul(pssq, lhsT=onesc, rhs=sqs[side, d],
                                         start=(d == 0), stop=(d == 1))
                    ssqs[side] = pssq
                for side in ("a", "b"):
                    nrm = wp.tile([1, A], F32, name=f"nrm{side}",
                                  tag=f"nrm{side}")
                    nc.scalar.sqrt(nrm, ssqs[side])
                    nrms[side] = nrm
                for side in ("a", "b"):
                    pbc = pss.tile([128, A], F32, name=f"bc{side}",
                                   tag="ps_small")
                    nc.tensor.matmul(pbc, lhsT=onesr, rhs=nrms[side],
                                     start=True, stop=True)
                    bcps[side] = pbc
                for side in ("a", "b"):
                    inv = wp.tile([128, A], F32, name=f"invb{side}",
                                  tag=f"invb{side}")
                    nc.vector.reciprocal(inv, bcps[side])
                    invs[side] = inv
                kna, knb = [], []
                for side, kT, lst in (("a", kaT, kna), ("b", kbT, knb)):
                    for d in range(2):
                        kn = wp.tile([128, A], F32, name=f"kn{side}{d}",
                                     tag=f"kn{side}{d}")
                        nc.vector.tensor_tensor(out=kn, in0=kT[d],
                                                in1=invs[side], op=AL.mult)
                        lst.append(kn)

                # scores[i, j] = kna[:, i] . knb[:, j]  -> [256 (2 tiles), 256]
                sc = []
                for m in range(2):
                    psc = pss.tile([128, A], F32, name="sc_ps", tag="ps_small")
                    for d in range(2):
                        nc.tensor.matmul(
                            psc, lhsT=kna[d][:, 128 * m:128 * (m + 1)],
                            rhs=knb[d], start=(d == 0), stop=(d == 1))
                    scs = wp.tile([128, A], F32, name=f"sc{m}", tag=f"sc{m}")
                    nc.vector.tensor_copy(scs, psc)
                    sc.append(scs)

                # Row max (best score) + argmax (first max index = dst).
                # dst lands in column 1 of pixrhs (iotap const in column 0)
                # so one [128,2] matmul later yields src-rows AND dstsel.
                best, dst, pixrhs = [], [], []
                for m in range(2):
                    bst = wp.tile([128, 1], F32, name=f"best{m}", tag=f"best{m}")
                    nc.vector.reduce_max(bst, sc[m], axis=AX.X)
                    best.append(bst)
                    pr = wp.tile([128, 2], F32, name=f"pixr{m}", tag=f"pixr{m}")
                    nc.vector.tensor_copy(pr[:, 0:1], iotap[m])
                    pixrhs.append(pr)
                for m in range(2):
                    am = wp.tile([128, A], F32, name="am", tag="am")
                    nc.vector.scalar_tensor_tensor(
                        out=am, in0=sc[m], scalar=best[m][:, :1], in1=iotam,
                        op0=AL.is_ge, op1=AL.mult)
                    dmin = wp.tile([128, 1], F32, name="dmin", tag="dmin")
                    nc.vector.tensor_reduce(dmin, am, axis=AX.X, op=AL.min)
                    dstt = wp.tile([128, 1], F32, name=f"dst{m}", tag=f"dst{m}")
                    nc.vector.tensor_scalar_add(dstt, dmin, BIG)
                    dst.append(dstt)

                # Broadcast best across partitions: bcb[p, j] = best[j].
                bcb = wp.tile([128, A], F32, name="bcb", tag="bcb")
                for m in range(2):
                    pbb = pss.tile([128, 128], F32, name="bb_ps", tag="ps_small")
                    nc.tensor.transpose(
                        out=pbb, in_=best[m][:, :1].to_broadcast([128, 128]),
                        identity=ident)
                    nc.vector.tensor_copy(bcb[:, 128 * m:128 * (m + 1)], pbb)

                # Dense rank with index tie-break:
                # rank_i = #{j: best_j > best_i} + #{j < i: best_j == best_i}
                # msk[:, 0] = selected (rank < R), msk[:, 1] = unmerged.
                msk = []
                for m in range(2):
                    eqlt = wp.tile([128, A], F32, name="eqlt", tag="eqlt")
                    nc.vector.scalar_tensor_tensor(
                        out=eqlt, in0=bcb, scalar=best[m][:, :1], in1=ltm[m],
                        op0=AL.is_equal, op1=AL.mult)
                    gt = wp.tile([128, A], F32, name="gt", tag="gt")
                    rank = wp.tile([128, 1], F32, name="rank", tag=f"rank{m}")
                    nc.vector.scalar_tensor_tensor(
                        out=gt, in0=bcb, scalar=best[m][:, :1], in1=eqlt,
                        op0=AL.is_gt, op1=AL.add, accum_out=rank)
                    mk = wp.tile([128, 2], F32, name=f"msk{m}", tag=f"msk{m}")
                    nc.vector.tensor_scalar(mk[:, 0:1], rank, float(R), None,
                                            op0=AL.is_lt)
                    nc.vector.tensor_scalar(mk[:, 1:2], rank, float(R), None,
                                            op0=AL.is_ge)
                    msk.append(mk)

                # Compaction prefix counts for both sets in one N=2 matmul:
                # col 0 = position among selected, col 1 = among unmerged.
                pos2 = []
                for m in range(2):
                    pps = pss.tile([128, 2], F32, name="pos_ps", tag="ps_small")
                    for kt in range(2):
                        nc.tensor.matmul(pps,
                                         lhsT=sut[kt][:, 128 * m:128 * (m + 1)],
                                         rhs=msk[kt], start=(kt == 0),
                                         stop=(kt == 1))
                    pst = wp.tile([128, 2], F32, name=f"pos2{m}", tag=f"pos2{m}")
                    nc.vector.tensor_copy(pst, pps)
                    pos2.append(pst)

                # One-hot compaction matrices (K-side layout).
                punmT, selT = [], []
                for kt in range(2):
                    pu = wp.tile([128, R], F32, name=f"punmT{kt}",
                                 tag=f"punmT{kt}")
                    nc.vector.tensor_scalar(pu, iota[:, :R],
                                            pos2[kt][:, 1:2],
                                            msk[kt][:, 1:2], op0=AL.is_equal,
                                            op1=AL.mult)
                    punmT.append(pu)
                    st = wp.tile([128, R], F32, name=f"selT{kt}",
                                 tag=f"selT{kt}")
                    nc.vector.tensor_scalar(st, iota[:, :R],
                                            pos2[kt][:, 0:1],
                                            msk[kt][:, 0:1], op0=AL.is_equal,
                                            op1=AL.mult)
                    selT.append(st)

                # Row indices + dstsel via two batched matmuls:
                #   selT.T @ [iotap | dst] -> [src a-idx | dstsel]
                #   punmT.T @ iotap       -> unm a-idx
                for m in range(2):
                    nc.vector.tensor_copy(pixrhs[m][:, 1:2], dst[m])
                psel = pss.tile([128, 2], F32, name="psel", tag="ps_small")
                for kt in range(2):
                    nc.tensor.matmul(psel, lhsT=selT[kt], rhs=pixrhs[kt],
                                     start=(kt == 0), stop=(kt == 1))
                punm = pss.tile([128, 1], F32, name="punm", tag="ps_small")
                for kt in range(2):
                    nc.tensor.matmul(punm, lhsT=punmT[kt],
                                     rhs=pixrhs[kt][:, 0:1],
                                     start=(kt == 0), stop=(kt == 1))

                srf = wp.tile([128, 1], F32, name="srf", tag="srf")
                nc.vector.tensor_scalar(srf, psel[:, 0:1], 2.0, float(r0),
                                        op0=AL.mult, op1=AL.add)
                srcrows = wp.tile([128, 1], I32, name="srcrows", tag="srcrows")
                nc.vector.tensor_copy(srcrows, srf)
                dstsel = wp.tile([128, 1], F32, name="dstsel", tag="dstsel")
                nc.vector.tensor_copy(dstsel, psel[:, 1:2])
                urf = wp.tile([128, 1], F32, name="urf", tag="urf")
                nc.vector.tensor_scalar(urf, punm, 2.0, float(r0),
                                        op0=AL.mult, op1=AL.add)
                unmrows = wp.tile([128, 1], I32, name="unmrows", tag="unmrows")
                nc.vector.tensor_copy(unmrows, urf)
                mselT = wp.tile([128, A], F32, name="mselT", tag="mselT")
                nc.vector.tensor_scalar(mselT, iota[:, :A], dstsel[:, :1], None,
                                        op0=AL.is_equal)

                win[w].update(mselT=mselT, punmT=punmT, selT=selT,
                              unmrows=unmrows, srcrows=srcrows)

            for w in range(WPC):
                r0 = W * w
                o0 = OUT_W * w
                st = win[w]
                xa, xb = st["xa"], st["xb"]
                mselT, punmT, selT = st["mselT"], st["punmT"], st["selT"]
                unmrows, srcrows = st["unmrows"], st["srcrows"]

                # ================= stage B: stream s =================
                bsnp = []
                for m in range(2):
                    bsnp.append(wp.tile([128, HPC * NCH], F32, name=f"bsnp{m}",
                                        tag=f"bsnp{m}"))
                ssp = wp.tile([128, NCH], F32, name="ssp", tag="ssp")

                for c in range(NCH):
                    c0 = CHUNK * c
                    unm_t = sp.tile([128, CHUNK], F32, name="unm_t",
                                    tag="unm_t", bufs=2)
                    nc.gpsimd.indirect_dma_start(
                        out=unm_t, out_offset=None, in_=s[:, :],
                        in_offset=bass.IndirectOffsetOnAxis(ap=unmrows[:, :1],
                                                            axis=0),
                        element_offset=c0)
                    src_t = sp.tile([128, CHUNK], F32, name="src_t",
                                    tag="src_t", bufs=3)
                    nc.gpsimd.indirect_dma_start(
                        out=src_t, out_offset=None, in_=s[:, :],
                        in_offset=bass.IndirectOffsetOnAxis(ap=srcrows[:, :1],
                                                            axis=0),
                        element_offset=c0)
                    # unmerged rows go straight out
                    nc.scalar.dma_start(out=out_s[o0:o0 + R, c0:c0 + CHUNK],
                                        in_=unm_t)
                    # selected-row sizes (row-sum partials)
                    nc.vector.reduce_sum(ssp[:, c:c + 1], src_t, axis=AX.X)

                    bs_t = []
                    for m in range(2):
                        rb = r0 + 256 * m
                        tb = sp.tile([128, CHUNK], F32, name=f"bst{m}",
                                     tag=f"bst{m}", bufs=8)
                        nc.sync.dma_start(out=tb,
                                          in_=s[rb + 1:rb + 256:2, c0:c0 + CHUNK])
                        bs_t.append(tb)

                    for m in range(2):
                        outb = sp.tile([128, CHUNK], F32, name=f"outb{m}",
                                       tag=f"outb{m}", bufs=3)
                        for h in range(HPC):
                            hs = 512 * h
                            pB = psb.tile([128, 512], F32, name="pB", tag="pB")
                            nc.tensor.matmul(
                                pB, lhsT=mselT[:, 128 * m:128 * (m + 1)],
                                rhs=src_t[:, hs:hs + 512], start=True, stop=True)
                            # fused (pB + 0) + bs with row-sum accumulator
                            # (tensor_tensor_reduce hard-crashes TRN2 here;
                            # scalar_tensor_tensor is the working fused form)
                            nc.vector.scalar_tensor_tensor(
                                out=outb[:, hs:hs + 512], in0=pB,
                                scalar=0.0, in1=bs_t[m][:, hs:hs + 512],
                                op0=AL.add, op1=AL.add,
                                accum_out=bsnp[m][:, HPC * c + h:HPC * c + h + 1])
                        ro = o0 + R + 128 * m
                        nc.scalar.dma_start(out=out_s[ro:ro + 128, c0:c0 + CHUNK],
                                            in_=outb)

                # ================= stage C: x merge =================
                srcsize = wp.tile([128, 1], F32, name="srcsize", tag="srcsize")
                nc.vector.reduce_sum(srcsize, ssp, axis=AX.X)
                bsnew = []
                for m in range(2):
                    bn = wp.tile([128, 1], F32, name=f"bsnew{m}", tag=f"bsnew{m}")
                    nc.vector.reduce_sum(bn, bsnp[m], axis=AX.X)
                    bsnew.append(bn)

                # selected x rows (compact order), weighted by size
                psx = pss.tile([128, D], F32, name="psx", tag="ps_small")
                for kt in range(2):
                    nc.tensor.matmul(psx, lhsT=selT[kt], rhs=xa[kt],
                                     start=(kt == 0), stop=(kt == 1))
                wsrc = wp.tile([128, D + 1], F32, name="wsrc", tag="wsrc")
                nc.scalar.mul(wsrc[:, :D], psx, srcsize[:, :1])
                nc.vector.tensor_copy(wsrc[:, D:D + 1], srcsize)

                # unmerged-a rows of out_x: plain gather via punmT
                pxa = pss.tile([128, D], F32, name="pxa", tag="ps_small")
                for kt in range(2):
                    nc.tensor.matmul(pxa, lhsT=punmT[kt], rhs=xa[kt],
                                     start=(kt == 0), stop=(kt == 1))
                outxa = wp.tile([128, D], F32, name="outxa", tag="outxa")
                nc.scalar.copy(outxa, pxa)
                nc.scalar.dma_start(out=out_x[o0:o0 + R, :], in_=outxa)

                # merged-b rows: (xb*b_size + Msel @ (srcx*srcsize)) / b_size_new
                for m in range(2):
                    pXB = psb.tile([128, D + 1], F32, name="pB", tag="pB")
                    nc.tensor.matmul(pXB,
                                     lhsT=mselT[:, 128 * m:128 * (m + 1)],
                                     rhs=wsrc, start=True, stop=True)
                    # b_size = b_size_new - (scattered src sizes)
                    bsz = wp.tile([128, 1], F32, name="bsz", tag="bsz")
                    nc.vector.tensor_tensor(out=bsz, in0=bsnew[m],
                                            in1=pXB[:, D:D + 1], op=AL.subtract)
                    wxb = wp.tile([128, D], F32, name="wxb", tag="wxb")
                    nc.scalar.mul(wxb, xb[m], bsz[:, :1])
                    numb = wp.tile([128, D], F32, name="numb", tag="numb")
                    nc.vector.tensor_tensor(out=numb, in0=pXB[:, :D], in1=wxb,
                                            op=AL.add)
                    den = wp.tile([128, 1], F32, name="den", tag="den")
                    nc.vector.tensor_scalar_max(den, bsnew[m], EPS)
                    rec = wp.tile([128, 1], F32, name="rec", tag="rec")
                    nc.vector.reciprocal(rec, den)
                    outxb = wp.tile([128, D], F32, name="outxb", tag="outxb")
                    nc.scalar.mul(outxb, numb, rec[:, :1])
                    ro = o0 + R + 128 * m
                    nc.scalar.dma_start(out=out_x[ro:ro + 128, :], in_=outxb)

    nc.compile()
    return nc


def host_constants(Wk: np.ndarray) -> dict[str, np.ndarray]:
    i = np.arange(A, dtype=np.float32)
    pk = np.zeros((128, C_TOT), dtype=np.float32)
    pk[:, C_ID:C_ID + 128] = np.eye(128, dtype=np.float32)
    pk[:, C_IOTA:C_IOTA + 512] = np.arange(512, dtype=np.float32)[None, :]
    pk[:, C_IOTAM:C_IOTAM + 256] = (np.arange(256, dtype=np.float32)
                                    - BIG)[None, :]
    sut = (i[:, None] < i[None, :]).astype(np.float32)   # sut[j,i]=1 if j<i
    ltm = (i[None, :] < i[:, None]).astype(np.float32)   # ltm[i,j]=1 if j<i
    pk[:, C_SUT:C_SUT + 256] = sut[0:128]
    pk[:, C_SUT + 256:C_SUT + 512] = sut[128:256]
    pk[:, C_LTM:C_LTM + 256] = ltm[0:128]
    pk[:, C_LTM + 256:C_LTM + 512] = ltm[128:256]
    Wk = np.asarray(Wk, np.float32)
    pk[:, C_WK:C_WK + 256] = Wk[0:128]
    pk[:, C_WK + 256:C_WK + 512] = Wk[128:256]
    pk[:, C_IOTAP] = np.arange(128, dtype=np.float32)
    pk[:, C_IOTAP + 1] = 128.0 + np.arange(128, dtype=np.float32)
    pk[:, C_ONES:C_ONES + 128] = 1.0
    return {"cpack": pk}


_CACHED_NC = None


def kernel(x: np.ndarray, s: np.ndarray, Wk: np.ndarray):
    from concourse.bass_utils import run_bass_kernel_spmd

    global _CACHED_NC
    if _CACHED_NC is None:
        _CACHED_NC = build_program()
    nc = _CACHED_NC

    x = np.ascontiguousarray(np.asarray(x, dtype=np.float32))
    s = np.ascontiguousarray(np.asarray(s, dtype=np.float32))
    consts = host_constants(Wk)

    in_maps = []
    for c in range(NCORES):
        r = ROWS_PC * c
        in_maps.append({
            "x": x[r:r + ROWS_PC],
            "xt": np.ascontiguousarray(x[r:r + ROWS_PC].T),
            "s": s[r:r + ROWS_PC],
            **consts,
        })

    res = run_bass_kernel_spmd(nc, in_maps, list(range(NCORES)))
    new_x = np.concatenate([res.results[c]["out_x"] for c in range(NCORES)],
                           axis=0)
    new_s = np.concatenate([res.results[c]["out_s"] for c in range(NCORES)],
                           axis=0)
    return new_x, new_s
